# revision 22
# baseline (speedup 1.0000x reference)
"""Trainium2 Bass kernel v3 for nn_Attention_14190571946482.

Causal self-attention (diagonal masked too), with both projection folds:
  B[d',d]   = sum_u Wv[d',u] Wq[d,u]          (device, 16 mm)
  u_cT[d,k] = sum_d' B[d',d] xT[d',k]         (replaces kT; scores become
  scoreT[k,q] = u_cT . xq / sqrt(D)            x A x^T - qT projection gone)
  ctxdT[d,q] = sum_c x_c[k,d]^T attnT_c[k,q]  (context in the d-basis -
  out[q,u]  = ctxdT^T @ Wk / den               v projection gone)

x_c chunk tiles ([k, d] layout) are PE-transposed from xT during phase 1
rather than DMA'd: the folds cut phase-1 PE work below the 2-queue DMA
feed rate, so input bytes are the binding resource (a late arrival also
risks the >~3us PE gap that resets the p-state, measured ~+18us).

Phase 2 is software-pipelined: scores/exp for chunk c+1 are emitted ahead
of den/ctxdT for chunk c (absorbs the exp->den latency), and each
finished slot's Wk-GEMM trails one further chunk so its cds copies
(vector) never stall the in-order PE queue.

Sharding: 8 cores = 4 batches x 2 roles; role r owns tiles {2j+r}.
Per core 2 groups of 4 slots: G2 = tiles {8..15} (chunks 0..15) first,
then G1 = tiles {0..7} (chunks 0..7). Role-dependent structure is input
data (qx gather, mask blocks, misc columns); the program is
SPMD-identical. Row 0 (fully masked) is blended to mean(v) on the final
psum before normalize.
"""

import sys

sys.path.insert(0, "/opt/trn_rl_repo")

import numpy as np
import ml_dtypes

import concourse.bass as bass
import concourse.bacc as bacc
import concourse.mybir as mybir
from concourse.tile import TileContext
from concourse.masks import make_identity
from concourse import bass_utils

BF16 = ml_dtypes.bfloat16

B, S, D, U = 4, 2048, 512, 512
P = 128
SCALE = 1.0 / float(np.sqrt(np.float32(D)))
GROUPS = [(8, 16), (0, 8)]  # (tbase, nchunks): G2 first, then G1
NSLOT = 8                   # output blocks: b=0..3 G2 slots, 4..7 G1
SPECIAL = 4                 # G1 slot 0 holds tiles (0,1): row-0 blend
NWARM = 10

_nc_cache = None


def build_nc():
    global _nc_cache
    if _nc_cache is not None:
        return _nc_cache

    f32 = mybir.dt.float32
    bf16 = mybir.dt.bfloat16

    nc = bacc.Bacc()
    xT_d = nc.declare_dram_parameter("xT", [D, S], bf16, isOutput=False)
    qx_d = nc.declare_dram_parameter("qx", [D, NSLOT * P], bf16, isOutput=False)
    wqT_d = nc.declare_dram_parameter("wqT", [U, D], bf16, isOutput=False)
    wvT_d = nc.declare_dram_parameter("wvT", [U, D], bf16, isOutput=False)
    wk_d = nc.declare_dram_parameter("wk", [D, U], bf16, isOutput=False)
    # 16 frontier mask blocks [128,128]: G2 chunks 8..15, then G1 0..7.
    mm_d = nc.declare_dram_parameter("maskblk", [P, 16 * P], bf16, isOutput=False)
    # misc f32: [0,0] rsel0 (row-0 ctx factor), [0,1] rscale (1/S or 0),
    # cols 8..15: per-output-block sume column.
    ms_d = nc.declare_dram_parameter("misc", [P, 16], f32, isOutput=False)
    out_d = nc.declare_dram_parameter("out", [NSLOT * P, U], bf16, isOutput=True)

    with TileContext(nc) as tc:
        with (
            tc.tile_pool(name="cst", bufs=1) as cst,
            tc.tile_pool(name="work", bufs=6) as work,
            tc.tile_pool(name="small", bufs=8) as small,
            tc.tile_pool(name="psA", bufs=2, space="PSUM") as psA,
            tc.tile_pool(name="psT", bufs=1, space="PSUM") as psT,
            tc.tile_pool(name="psC", bufs=4, space="PSUM") as psC,
            tc.tile_pool(name="psD", bufs=1, space="PSUM") as psD,
        ):
            # ---- on-chip constants ----
            # warm-up operand: iota, NOT zeros — the power governor keys
            # on multiplier bit-toggling, and zero x zero never trips the
            # clock boost (measured: HAM start 25us with zeros)
            wu = cst.tile([P, 512], bf16, tag="wu")
            nc.gpsimd.iota(wu, pattern=[[1, 512]], base=1,
                           channel_multiplier=7,
                           allow_small_or_imprecise_dtypes=True)
            ones_c = cst.tile([P, 1], bf16, tag="ones")
            nc.gpsimd.memset(ones_c, 1.0)
            ident = cst.tile([P, P], bf16, tag="ident")
            make_identity(nc, ident)

            # ---- input DMAs on the two HW-DGE issue queues (~150-180GB/s
            # each; gpsimd DMA is the slow SWDGE path - do not use).
            # qx[:512] covers all of G2; qx[512:] is G1-only (~40us in).
            # misc gates the first den add (~28us), maskblk the first
            # frontier chunk (~28us). ----
            wvT_t = cst.tile([P, 4, D], bf16, tag="wvT")
            wqT_t = cst.tile([P, 4, D], bf16, tag="wqT")
            xT_t = cst.tile([P, 4, S], bf16, tag="xT")
            xT_r = xT_d.rearrange("(d p) s -> p d s", p=P)
            wk_t = cst.tile([P, 4, U], bf16, tag="wk")
            qx_t = cst.tile([P, 4, NSLOT * P], bf16, tag="qx")
            qx_r = qx_d.rearrange("(d p) s -> p d s", p=P)
            maskblk = cst.tile([P, 16 * P], bf16, tag="maskblk")
            misc = cst.tile([P, 16], f32, tag="misc")
            nc.sync.dma_start(out=xT_t[:, :, 0:512], in_=xT_r[:, :, 0:512])
            nc.scalar.dma_start(out=wqT_t, in_=wqT_d.rearrange("(k p) d -> p k d", p=P))
            nc.sync.dma_start(out=wvT_t, in_=wvT_d.rearrange("(k p) d -> p k d", p=P))
            nc.scalar.dma_start(out=xT_t[:, :, 512:1024], in_=xT_r[:, :, 512:1024])
            nc.sync.dma_start(out=xT_t[:, :, 1024:1536], in_=xT_r[:, :, 1024:1536])
            nc.scalar.dma_start(out=xT_t[:, :, 1536:2048], in_=xT_r[:, :, 1536:2048])
            nc.sync.dma_start(out=qx_t[:, :, 0:512], in_=qx_r[:, :, 0:512])
            nc.scalar.dma_start(out=wk_t, in_=wk_d.rearrange("(d p) u -> p d u", p=P))
            nc.sync.dma_start(out=misc, in_=ms_d[:, :])
            nc.sync.dma_start(out=maskblk, in_=mm_d[:, :])
            nc.scalar.dma_start(out=qx_t[:, :, 512:1024], in_=qx_r[:, :, 512:1024])

            # ---- PE warm-up: ramp the HAM clock while DMAs land (also
            # bridges the PE to the wvT/wqT arrival ~13us) ----
            dume = small.tile([1, 1], bf16, tag="dume")
            for w in range(NWARM):
                wups = psA.tile([P, 512], f32, tag="blk")
                nc.tensor.matmul(wups, lhsT=wu[:, :P], rhs=wu,
                                 start=True, stop=True)
                if w == 0:
                    # preload the scalar-engine exp table off-critical-path
                    nc.scalar.activation(
                        dume, wups[0:1, 0:1],
                        mybir.ActivationFunctionType.Exp, scale=SCALE)

            uT = [cst.tile([P, S], bf16, tag=f"uT{m}", name=f"uT{m}")
                  for m in range(4)]
            xo_t = cst.tile([P, 16, D], bf16, tag="xo")

            def emit_quad(c):
                tq = psT.tile([P, 4, P], bf16, tag="tq")
                for mm in range(4):
                    nc.tensor.transpose(
                        tq[:, mm, :], xT_t[:, mm, c * P:(c + 1) * P], ident)
                    dst = xo_t[:, c, mm * P:(mm + 1) * P]
                    if (c + mm) % 2 == 0:
                        nc.scalar.copy(dst, tq[:, mm, :])
                    else:
                        nc.vector.tensor_copy(dst, tq[:, mm, :])

            # ---- transposes of x chunks 0..3 ([k, d] layout): need only
            # the first xT slice, so they keep the PE busy (and the clock
            # governor ramping) while wvT/wqT are still in flight ----
            for c in range(4):
                emit_quad(c)

            # ---- B = Wv @ Wq^T, tiles [d' part, d free] ----
            B_sb = cst.tile([P, 4, D], bf16, tag="Bsb")
            for t in range(4):
                ps = psA.tile([P, 512], f32, tag="blk")
                for ku in range(4):
                    nc.tensor.matmul(
                        ps,
                        lhsT=wvT_t[:, ku, t * P:(t + 1) * P],
                        rhs=wqT_t[:, ku, :],
                        start=(ku == 0), stop=(ku == 3),
                    )
                if t % 2 == 0:
                    nc.vector.tensor_copy(B_sb[:, t, :], ps)
                else:
                    nc.scalar.copy(B_sb[:, t, :], ps)

            # ---- u_cT [d, s] per g-slice (follows the xT DMA) ----
            ci = 0
            for g in range(4):
                if g in (1, 2):
                    # DMA-jitter insurance: independent filler matmuls so a
                    # late xT slice never leaves the PE (and the clock
                    # governor) idle
                    for _ in range(2):
                        wf = psA.tile([P, 512], f32, tag="blk")
                        nc.tensor.matmul(wf, lhsT=wu[:, :P], rhs=wu,
                                         start=True, stop=True)
                for m in range(4):
                    ps = psA.tile([P, 512], f32, tag="blk")
                    for t in range(4):
                        nc.tensor.matmul(
                            ps,
                            lhsT=B_sb[:, t, m * P:(m + 1) * P],
                            rhs=xT_t[:, t, g * 512:(g + 1) * 512],
                            start=(t == 0), stop=(t == 3),
                        )
                    dst = uT[m][:, g * 512:(g + 1) * 512]
                    if ci % 2 == 0:
                        nc.vector.tensor_copy(dst, ps)
                    else:
                        nc.scalar.copy(dst, ps)
                    ci += 1
                    # transpose quad for chunk c = 4g+m (g0's ran pre-B);
                    # its copies drain during the next m-iteration
                    if g > 0:
                        emit_quad(4 * g + m)

            # ---- phase 2: transposed-score attention, d-basis context ----
            vm_sb = cst.tile([1, 512], f32, tag="vm_sb")
            # cols 0..7: slot denominators; cols 8..11: per-d-tile column
            # sums of x (for mean-v), accumulated like den via ones-matmuls
            den_t = psD.tile([P, 12], f32, tag="dent", name="dent")
            nc.vector.memset(den_t, 0.0)

            def emit_back(st):
                """den + ctxdT (+slot-finish prep) for a staged chunk."""
                g, tbase, c, nchunks, j0, ncols, attnT, cd_ps = st
                cl = c - tbase
                fin = (cl % 2 == 1) and cl >= 0
                jf = cl // 2 if fin else -1
                js = list(range(j0, 4))
                if cl >= 0:
                    js = js[1:] + js[:1]  # masked slot j0 last
                for j in js:
                    b = 4 * g + j
                    blk = attnT[:, (j - j0) * P:(j - j0 + 1) * P]
                    nc.tensor.matmul(den_t[:, b:b + 1], lhsT=blk,
                                     rhs=ones_c, start=False,
                                     stop=(j == jf),
                                     skip_group_check=True)
                for m in range(4):
                    nc.tensor.matmul(
                        cd_ps[m][:, j0 * P:512],
                        lhsT=xo_t[:, c, m * P:(m + 1) * P],
                        rhs=attnT[:, :ncols],
                        start=(c == 0), stop=(c == nchunks - 1),
                        skip_group_check=True,
                    )
                if g == 0:
                    for m in range(4):
                        nc.tensor.matmul(den_t[:, 8 + m:9 + m],
                                         lhsT=xo_t[:, c, m * P:(m + 1) * P],
                                         rhs=ones_c, start=False,
                                         stop=(c == nchunks - 1),
                                         skip_group_check=True)
                if not fin:
                    return None
                j = jf
                b = 4 * g + j
                den = small.tile([P, 1], f32, tag="den")
                nc.vector.tensor_add(den, den_t[:, b:b + 1],
                                     misc[:, 8 + b:9 + b])
                rcp = small.tile([P, 1], f32, tag="rcp")
                nc.vector.reciprocal(rcp, den)
                # cds copies on vector only: the scalar queue must stay
                # clear for the next chunks' exps
                cds = work.tile([P, 4, P], bf16, tag="cds")
                for m in range(4):
                    nc.vector.tensor_copy(cds[:, m, :],
                                          cd_ps[m][:, j * P:(j + 1) * P])
                return (b, cds, rcp)

            def emit_gemm(st):
                """Wk-GEMM + normalize + out DMA for a finished slot."""
                b, cds, rcp = st
                out_ps = psA.tile([P, 512], f32, tag="blk")
                for m in range(4):
                    nc.tensor.matmul(out_ps, lhsT=cds[:, m, :],
                                     rhs=wk_t[:, m, :],
                                     start=(m == 0), stop=(m == 3))
                if b == SPECIAL:
                    # row 0 of role 0 = mean(v): on psum f32
                    nc.vector.tensor_scalar_mul(
                        out_ps[0:1, :], out_ps[0:1, :], misc[0:1, 0:1])
                    nc.vector.tensor_add(out_ps[0:1, :], out_ps[0:1, :],
                                         vm_sb)
                ctx_sb = work.tile([P, 512], bf16, tag="ctxs")
                for hh in range(4):
                    nc.scalar.activation(
                        ctx_sb[:, hh * 128:(hh + 1) * 128],
                        out_ps[:, hh * 128:(hh + 1) * 128],
                        mybir.ActivationFunctionType.Copy,
                        scale=rcp)
                    nc.sync.dma_start(
                        out=out_d[b * P:(b + 1) * P,
                                  hh * 128:(hh + 1) * 128],
                        in_=ctx_sb[:, hh * 128:(hh + 1) * 128])

            # flat chunk stream across both groups: the staged/pending
            # pipeline carries over the G2->G1 boundary so the PE never
            # drains at the group switch
            cd_pool = {}
            staged = None
            pend_gemm = None
            chunk_stream = [(g, tbase, nchunks, c)
                            for g, (tbase, nchunks) in enumerate(GROUPS)
                            for c in range(nchunks)]
            for g, tbase, nchunks, c in chunk_stream:
                if g == 1 and c == 0:
                    # ---- mean-of-v (for the fully-masked global row 0):
                    # needed first at the SPECIAL slot (G1 cl=1). The
                    # x column sums accumulated on PE during G2 (vector
                    # reduces get hoisted by the Tile scheduler into the
                    # phase-1 stream where they delay the uT copies) ----
                    xs4 = small.tile([P, 4], bf16, tag="xs4")
                    nc.vector.tensor_copy(xs4, den_t[:, 8:12])
                    vm_ps = psA.tile([1, 512], f32, tag="blk")
                    for d in range(4):
                        nc.tensor.matmul(vm_ps, lhsT=xs4[:, d:d + 1],
                                         rhs=wk_t[:, d, :],
                                         start=(d == 0), stop=(d == 3))
                    nc.vector.tensor_scalar_mul(vm_sb, vm_ps, misc[0:1, 1:2])
                if c == 0:
                    cd_pool[g] = [psC.tile([P, 512], f32, tag="ctx",
                                           name=f"cd{g}_{m}")
                                  for m in range(4)]
                cd_ps = cd_pool[g]
                mask_base = 0 if g == 0 else 8
                j0 = max(0, (c - tbase) // 2)
                ncols = (4 - j0) * P
                qoff = g * 512 + j0 * P
                sc_ps = psA.tile([P, 512], f32, tag="blk")
                for m in range(4):
                    nc.tensor.matmul(
                        sc_ps[:, :ncols],
                        lhsT=uT[m][:, c * P:(c + 1) * P],
                        rhs=qx_t[:, m, qoff:qoff + ncols],
                        start=(m == 0), stop=(m == 3),
                    )
                attnT = work.tile([P, 512], bf16, tag="attnT")
                cl = c - tbase
                if cl >= 0:
                    # frontier chunk: the masked block is always the
                    # first live block (j == j0). Exp it first so the
                    # vector mask-mul overlaps the exp of the rest.
                    mb = (mask_base + cl) * P
                    nc.scalar.activation(
                        attnT[:, 0:P], sc_ps[:, 0:P],
                        mybir.ActivationFunctionType.Exp, scale=SCALE,
                    )
                    nc.vector.tensor_mul(attnT[:, 0:P], attnT[:, 0:P],
                                         maskblk[:, mb:mb + P])
                    if ncols > P:
                        nc.scalar.activation(
                            attnT[:, P:ncols], sc_ps[:, P:ncols],
                            mybir.ActivationFunctionType.Exp, scale=SCALE,
                        )
                else:
                    nc.scalar.activation(
                        attnT[:, :ncols], sc_ps[:, :ncols],
                        mybir.ActivationFunctionType.Exp, scale=SCALE,
                    )
                if pend_gemm is not None:
                    emit_gemm(pend_gemm)
                    pend_gemm = None
                if staged is not None:
                    pend_gemm = emit_back(staged)
                staged = (g, tbase, c, nchunks, j0, ncols, attnT, cd_ps)
            if pend_gemm is not None:
                emit_gemm(pend_gemm)
            pend_gemm = emit_back(staged)
            if pend_gemm is not None:
                emit_gemm(pend_gemm)

    nc.compile()
    _nc_cache = nc
    return nc


def tile_of_block(b, r):
    """Global q-tile held by output block b on role r."""
    return (8 + 2 * b + r) if b < 4 else (2 * (b - 4) + r)


def host_inputs(query, Wq, Wv, Wk):
    """Build per-core input maps. query [B,S,D] f32; W* [D,U] f32."""
    wqT16 = np.ascontiguousarray(Wq.T).astype(BF16)
    wvT16 = np.ascontiguousarray(Wv.T).astype(BF16)
    wk16 = Wk.astype(BF16)

    p = np.arange(P)[:, None]   # kk within chunk
    f = np.arange(P)[None, :]   # q within tile
    tri = (p < f).astype(np.float32)        # diag block: kk < q valid
    ones_b = np.ones((P, P), np.float32)
    zeros_b = np.zeros((P, P), np.float32)

    masks = {}
    for r in range(2):
        blocks = []
        for g, (tbase, nchunks) in enumerate(GROUPS):
            for cl in range(8):
                # chunk c = tbase + cl, affected slot j = cl//2,
                # role tile t = tbase + 2*(cl//2) + r
                c = tbase + cl
                t = tbase + 2 * (cl // 2) + r
                if c < t:
                    blocks.append(ones_b)
                elif c == t:
                    blocks.append(tri)
                else:
                    blocks.append(zeros_b)
        masks[r] = np.concatenate(blocks, axis=1).astype(BF16)

    in_maps = []
    for core in range(8):
        b_, r = core // 2, core % 2
        xTb = np.ascontiguousarray(query[b_].T).astype(BF16)      # [D, S]
        cols = np.concatenate(
            [np.arange(P * tile_of_block(b, r), P * tile_of_block(b, r) + P)
             for b in range(NSLOT)]
        )
        qx = np.ascontiguousarray(xTb[:, cols])                   # [D, 1024]
        misc = np.zeros((P, 16), np.float32)
        misc[0, 0] = 0.0 if r == 0 else 1.0      # rsel0
        misc[0, 1] = (1.0 / S) if r == 0 else 0.0  # rscale
        if r == 0:
            misc[0, 8 + SPECIAL] = 1.0           # den fix for global row 0
        in_maps.append({
            "xT": xTb, "qx": qx,
            "wqT": wqT16, "wvT": wvT16, "wk": wk16,
            "maskblk": masks[r], "misc": misc,
        })
    return in_maps


def assemble_output(results):
    """results: list of 8 dicts with 'out' [1024, 512] bf16."""
    out = np.zeros((B, S, U), np.float32)
    for core in range(8):
        b_, r = core // 2, core % 2
        o = np.asarray(results[core]["out"], dtype=np.float32)
        for b in range(NSLOT):
            t = tile_of_block(b, r)
            out[b_, P * t:P * (t + 1), :] = o[P * b:P * (b + 1), :]
    return out


def run(query, Wq, Wv, Wk, **kwargs):
    nc = build_nc()
    in_maps = host_inputs(
        np.asarray(query, np.float32), np.asarray(Wq, np.float32),
        np.asarray(Wv, np.float32), np.asarray(Wk, np.float32),
    )
    res = bass_utils.run_bass_kernel_spmd(nc, in_maps, list(range(8)), **kwargs)
    return assemble_output(res.results), res


def kernel(query, Wq, Wv, Wk):
    out, _ = run(query, Wq, Wv, Wk)
    return out


if __name__ == "__main__":
    rng = np.random.default_rng(0)
    q = rng.standard_normal((B, S, D), dtype=np.float32)
    scale = np.sqrt(2.0 / (D + U)).astype(np.float32)
    Wq = rng.standard_normal((D, U), dtype=np.float32) * scale
    Wv = rng.standard_normal((D, U), dtype=np.float32) * scale
    Wk = rng.standard_normal((D, U), dtype=np.float32) * scale
    out = kernel(q, Wq, Wv, Wk)
    print(out.shape, out.dtype, np.abs(out).mean())


# revision 23
# speedup vs baseline: 1.0035x; 1.0035x over previous
"""Trainium2 Bass kernel v3 for nn_Attention_14190571946482.

Causal self-attention (diagonal masked too), with both projection folds:
  B[d',d]   = sum_u Wv[d',u] Wq[d,u]          (device, 16 mm)
  u_cT[d,k] = sum_d' B[d',d] xT[d',k]         (replaces kT; scores become
  scoreT[k,q] = u_cT . xq / sqrt(D)            x A x^T - qT projection gone)
  ctxdT[d,q] = sum_c x_c[k,d]^T attnT_c[k,q]  (context in the d-basis -
  out[q,u]  = ctxdT^T @ Wk / den               v projection gone)

x_c chunk tiles ([k, d] layout) are PE-transposed from xT during phase 1
rather than DMA'd: the folds cut phase-1 PE work below the 2-queue DMA
feed rate, so input bytes are the binding resource (a late arrival also
risks the >~3us PE gap that resets the p-state, measured ~+18us).

Phase 2 is software-pipelined: scores/exp for chunk c+1 are emitted ahead
of den/ctxdT for chunk c (absorbs the exp->den latency), and each
finished slot's Wk-GEMM trails one further chunk so its cds copies
(vector) never stall the in-order PE queue.

Sharding: 8 cores = 4 batches x 2 roles; role r owns tiles {2j+r}.
Per core 2 groups of 4 slots: G2 = tiles {8..15} (chunks 0..15) first,
then G1 = tiles {0..7} (chunks 0..7). Role-dependent structure is input
data (qx gather, mask blocks, misc columns); the program is
SPMD-identical. Row 0 (fully masked) is blended to mean(v) on the final
psum before normalize.
"""

import sys

sys.path.insert(0, "/opt/trn_rl_repo")

import numpy as np
import ml_dtypes

import concourse.bass as bass
import concourse.bacc as bacc
import concourse.mybir as mybir
from concourse.tile import TileContext
from concourse.masks import make_identity
from concourse import bass_utils

BF16 = ml_dtypes.bfloat16

B, S, D, U = 4, 2048, 512, 512
P = 128
SCALE = 1.0 / float(np.sqrt(np.float32(D)))
GROUPS = [(8, 16), (0, 8)]  # (tbase, nchunks): G2 first, then G1
NSLOT = 8                   # output blocks: b=0..3 G2 slots, 4..7 G1
SPECIAL = 4                 # G1 slot 0 holds tiles (0,1): row-0 blend
NWARM = 10

_nc_cache = None


def build_nc():
    global _nc_cache
    if _nc_cache is not None:
        return _nc_cache

    f32 = mybir.dt.float32
    bf16 = mybir.dt.bfloat16

    nc = bacc.Bacc()
    xT_d = nc.declare_dram_parameter("xT", [D, S], bf16, isOutput=False)
    qx_d = nc.declare_dram_parameter("qx", [D, NSLOT * P], bf16, isOutput=False)
    wqT_d = nc.declare_dram_parameter("wqT", [U, D], bf16, isOutput=False)
    wvT_d = nc.declare_dram_parameter("wvT", [U, D], bf16, isOutput=False)
    wk_d = nc.declare_dram_parameter("wk", [D, U], bf16, isOutput=False)
    # 16 frontier mask blocks [128,128]: G2 chunks 8..15, then G1 0..7.
    mm_d = nc.declare_dram_parameter("maskblk", [P, 16 * P], bf16, isOutput=False)
    # misc f32: [0,0] rsel0 (row-0 ctx factor), [0,1] rscale (1/S or 0),
    # cols 8..15: per-output-block sume column.
    ms_d = nc.declare_dram_parameter("misc", [P, 16], f32, isOutput=False)
    out_d = nc.declare_dram_parameter("out", [NSLOT * P, U], bf16, isOutput=True)

    with TileContext(nc) as tc:
        with (
            tc.tile_pool(name="cst", bufs=1) as cst,
            tc.tile_pool(name="work", bufs=6) as work,
            tc.tile_pool(name="small", bufs=8) as small,
            tc.tile_pool(name="psA", bufs=2, space="PSUM") as psA,
            tc.tile_pool(name="psT", bufs=1, space="PSUM") as psT,
            tc.tile_pool(name="psC", bufs=4, space="PSUM") as psC,
            tc.tile_pool(name="psD", bufs=1, space="PSUM") as psD,
        ):
            # ---- on-chip constants ----
            # warm-up operand: iota, NOT zeros — the power governor keys
            # on multiplier bit-toggling, and zero x zero never trips the
            # clock boost (measured: HAM start 25us with zeros)
            wu = cst.tile([P, 512], bf16, tag="wu")
            nc.gpsimd.iota(wu, pattern=[[1, 512]], base=1,
                           channel_multiplier=7,
                           allow_small_or_imprecise_dtypes=True)
            ones_c = cst.tile([P, 1], bf16, tag="ones")
            nc.gpsimd.memset(ones_c, 1.0)
            ident = cst.tile([P, P], bf16, tag="ident")
            make_identity(nc, ident)

            # ---- input DMAs on the two HW-DGE issue queues (~150-180GB/s
            # each; gpsimd DMA is the slow SWDGE path - do not use).
            # qx[:512] covers all of G2; qx[512:] is G1-only (~40us in).
            # misc gates the first den add (~28us), maskblk the first
            # frontier chunk (~28us). ----
            wvT_t = cst.tile([P, 4, D], bf16, tag="wvT")
            wqT_t = cst.tile([P, 4, D], bf16, tag="wqT")
            xT_t = cst.tile([P, 4, S], bf16, tag="xT")
            xT_r = xT_d.rearrange("(d p) s -> p d s", p=P)
            wk_t = cst.tile([P, 4, U], bf16, tag="wk")
            qx_t = cst.tile([P, 4, NSLOT * P], bf16, tag="qx")
            qx_r = qx_d.rearrange("(d p) s -> p d s", p=P)
            maskblk = cst.tile([P, 16 * P], bf16, tag="maskblk")
            misc = cst.tile([P, 16], f32, tag="misc")
            nc.sync.dma_start(out=wvT_t, in_=wvT_d.rearrange("(k p) d -> p k d", p=P))
            nc.scalar.dma_start(out=wqT_t, in_=wqT_d.rearrange("(k p) d -> p k d", p=P))
            nc.sync.dma_start(out=xT_t[:, :, 0:512], in_=xT_r[:, :, 0:512])
            nc.scalar.dma_start(out=xT_t[:, :, 512:1024], in_=xT_r[:, :, 512:1024])
            nc.sync.dma_start(out=xT_t[:, :, 1024:1536], in_=xT_r[:, :, 1024:1536])
            nc.scalar.dma_start(out=xT_t[:, :, 1536:2048], in_=xT_r[:, :, 1536:2048])
            nc.sync.dma_start(out=qx_t[:, :, 0:512], in_=qx_r[:, :, 0:512])
            nc.scalar.dma_start(out=wk_t, in_=wk_d.rearrange("(d p) u -> p d u", p=P))
            nc.sync.dma_start(out=misc, in_=ms_d[:, :])
            nc.sync.dma_start(out=maskblk, in_=mm_d[:, :])
            nc.scalar.dma_start(out=qx_t[:, :, 512:1024], in_=qx_r[:, :, 512:1024])

            # ---- PE warm-up: ramp the HAM clock while DMAs land (also
            # bridges the PE to the wvT/wqT arrival ~13us) ----
            dume = small.tile([1, 1], bf16, tag="dume")
            for w in range(NWARM):
                wups = psA.tile([P, 512], f32, tag="blk")
                nc.tensor.matmul(wups, lhsT=wu[:, :P], rhs=wu,
                                 start=True, stop=True)
                if w == 0:
                    # preload the scalar-engine exp table off-critical-path
                    nc.scalar.activation(
                        dume, wups[0:1, 0:1],
                        mybir.ActivationFunctionType.Exp, scale=SCALE)

            uT = [cst.tile([P, S], bf16, tag=f"uT{m}", name=f"uT{m}")
                  for m in range(4)]
            xo_t = cst.tile([P, 16, D], bf16, tag="xo")

            def emit_quad(c):
                tq = psT.tile([P, 4, P], bf16, tag="tq")
                for mm in range(4):
                    nc.tensor.transpose(
                        tq[:, mm, :], xT_t[:, mm, c * P:(c + 1) * P], ident)
                    dst = xo_t[:, c, mm * P:(mm + 1) * P]
                    if (c + mm) % 2 == 0:
                        nc.scalar.copy(dst, tq[:, mm, :])
                    else:
                        nc.vector.tensor_copy(dst, tq[:, mm, :])

            # ---- transposes of x chunks 0..3 ([k, d] layout): need only
            # the first xT slice, so they keep the PE busy (and the clock
            # governor ramping) while wvT/wqT are still in flight ----
            for c in range(4):
                emit_quad(c)

            # ---- B = Wv @ Wq^T, tiles [d' part, d free] ----
            B_sb = cst.tile([P, 4, D], bf16, tag="Bsb")
            for t in range(4):
                ps = psA.tile([P, 512], f32, tag="blk")
                for ku in range(4):
                    nc.tensor.matmul(
                        ps,
                        lhsT=wvT_t[:, ku, t * P:(t + 1) * P],
                        rhs=wqT_t[:, ku, :],
                        start=(ku == 0), stop=(ku == 3),
                    )
                if t % 2 == 0:
                    nc.vector.tensor_copy(B_sb[:, t, :], ps)
                else:
                    nc.scalar.copy(B_sb[:, t, :], ps)

            # ---- u_cT [d, s] per g-slice (follows the xT DMA) ----
            ci = 0
            for g in range(4):
                if g in (1, 2):
                    # DMA-jitter insurance: independent filler matmuls so a
                    # late xT slice never leaves the PE (and the clock
                    # governor) idle
                    for _ in range(2):
                        wf = psA.tile([P, 512], f32, tag="blk")
                        nc.tensor.matmul(wf, lhsT=wu[:, :P], rhs=wu,
                                         start=True, stop=True)
                for m in range(4):
                    ps = psA.tile([P, 512], f32, tag="blk")
                    for t in range(4):
                        nc.tensor.matmul(
                            ps,
                            lhsT=B_sb[:, t, m * P:(m + 1) * P],
                            rhs=xT_t[:, t, g * 512:(g + 1) * 512],
                            start=(t == 0), stop=(t == 3),
                        )
                    dst = uT[m][:, g * 512:(g + 1) * 512]
                    if ci % 2 == 0:
                        nc.vector.tensor_copy(dst, ps)
                    else:
                        nc.scalar.copy(dst, ps)
                    ci += 1
                    # transpose quad for chunk c = 4g+m (g0's ran pre-B);
                    # its copies drain during the next m-iteration
                    if g > 0:
                        emit_quad(4 * g + m)

            # ---- phase 2: transposed-score attention, d-basis context ----
            vm_sb = cst.tile([1, 512], f32, tag="vm_sb")
            # cols 0..7: slot denominators; cols 8..11: per-d-tile column
            # sums of x (for mean-v), accumulated like den via ones-matmuls
            den_t = psD.tile([P, 12], f32, tag="dent", name="dent")
            nc.vector.memset(den_t, 0.0)

            def emit_back(st):
                """den + ctxdT (+slot-finish prep) for a staged chunk."""
                g, tbase, c, nchunks, j0, ncols, attnT, cd_ps = st
                cl = c - tbase
                fin = (cl % 2 == 1) and cl >= 0
                jf = cl // 2 if fin else -1
                js = list(range(j0, 4))
                if cl >= 0:
                    js = js[1:] + js[:1]  # masked slot j0 last
                for j in js:
                    b = 4 * g + j
                    blk = attnT[:, (j - j0) * P:(j - j0 + 1) * P]
                    nc.tensor.matmul(den_t[:, b:b + 1], lhsT=blk,
                                     rhs=ones_c, start=False,
                                     stop=(j == jf),
                                     skip_group_check=True)
                for m in range(4):
                    nc.tensor.matmul(
                        cd_ps[m][:, j0 * P:512],
                        lhsT=xo_t[:, c, m * P:(m + 1) * P],
                        rhs=attnT[:, :ncols],
                        start=(c == 0), stop=(c == nchunks - 1),
                        skip_group_check=True,
                    )
                if g == 0:
                    for m in range(4):
                        nc.tensor.matmul(den_t[:, 8 + m:9 + m],
                                         lhsT=xo_t[:, c, m * P:(m + 1) * P],
                                         rhs=ones_c, start=False,
                                         stop=(c == nchunks - 1),
                                         skip_group_check=True)
                if not fin:
                    return None
                j = jf
                b = 4 * g + j
                den = small.tile([P, 1], f32, tag="den")
                nc.vector.tensor_add(den, den_t[:, b:b + 1],
                                     misc[:, 8 + b:9 + b])
                rcp = small.tile([P, 1], f32, tag="rcp")
                nc.vector.reciprocal(rcp, den)
                # cds copies on vector only: the scalar queue must stay
                # clear for the next chunks' exps
                cds = work.tile([P, 4, P], bf16, tag="cds")
                for m in range(4):
                    nc.vector.tensor_copy(cds[:, m, :],
                                          cd_ps[m][:, j * P:(j + 1) * P])
                return (b, cds, rcp)

            def emit_gemm(st):
                """Wk-GEMM + normalize + out DMA for a finished slot."""
                b, cds, rcp = st
                out_ps = psA.tile([P, 512], f32, tag="blk")
                for m in range(4):
                    nc.tensor.matmul(out_ps, lhsT=cds[:, m, :],
                                     rhs=wk_t[:, m, :],
                                     start=(m == 0), stop=(m == 3))
                if b == SPECIAL:
                    # row 0 of role 0 = mean(v): on psum f32
                    nc.vector.tensor_scalar_mul(
                        out_ps[0:1, :], out_ps[0:1, :], misc[0:1, 0:1])
                    nc.vector.tensor_add(out_ps[0:1, :], out_ps[0:1, :],
                                         vm_sb)
                ctx_sb = work.tile([P, 512], bf16, tag="ctxs")
                for hh in range(4):
                    nc.scalar.activation(
                        ctx_sb[:, hh * 128:(hh + 1) * 128],
                        out_ps[:, hh * 128:(hh + 1) * 128],
                        mybir.ActivationFunctionType.Copy,
                        scale=rcp)
                    nc.sync.dma_start(
                        out=out_d[b * P:(b + 1) * P,
                                  hh * 128:(hh + 1) * 128],
                        in_=ctx_sb[:, hh * 128:(hh + 1) * 128])

            # flat chunk stream across both groups: the staged/pending
            # pipeline carries over the G2->G1 boundary so the PE never
            # drains at the group switch
            cd_pool = {}
            staged = None
            pend_gemm = None
            chunk_stream = [(g, tbase, nchunks, c)
                            for g, (tbase, nchunks) in enumerate(GROUPS)
                            for c in range(nchunks)]
            for g, tbase, nchunks, c in chunk_stream:
                if g == 1 and c == 0:
                    # ---- mean-of-v (for the fully-masked global row 0):
                    # needed first at the SPECIAL slot (G1 cl=1). The
                    # x column sums accumulated on PE during G2 (vector
                    # reduces get hoisted by the Tile scheduler into the
                    # phase-1 stream where they delay the uT copies) ----
                    xs4 = small.tile([P, 4], bf16, tag="xs4")
                    nc.vector.tensor_copy(xs4, den_t[:, 8:12])
                    vm_ps = psA.tile([1, 512], f32, tag="blk")
                    for d in range(4):
                        nc.tensor.matmul(vm_ps, lhsT=xs4[:, d:d + 1],
                                         rhs=wk_t[:, d, :],
                                         start=(d == 0), stop=(d == 3))
                    nc.vector.tensor_scalar_mul(vm_sb, vm_ps, misc[0:1, 1:2])
                if c == 0:
                    cd_pool[g] = [psC.tile([P, 512], f32, tag="ctx",
                                           name=f"cd{g}_{m}")
                                  for m in range(4)]
                cd_ps = cd_pool[g]
                mask_base = 0 if g == 0 else 8
                j0 = max(0, (c - tbase) // 2)
                ncols = (4 - j0) * P
                qoff = g * 512 + j0 * P
                sc_ps = psA.tile([P, 512], f32, tag="blk")
                for m in range(4):
                    nc.tensor.matmul(
                        sc_ps[:, :ncols],
                        lhsT=uT[m][:, c * P:(c + 1) * P],
                        rhs=qx_t[:, m, qoff:qoff + ncols],
                        start=(m == 0), stop=(m == 3),
                    )
                attnT = work.tile([P, 512], bf16, tag="attnT")
                cl = c - tbase
                if cl >= 0:
                    # frontier chunk: the masked block is always the
                    # first live block (j == j0). Exp it first so the
                    # vector mask-mul overlaps the exp of the rest.
                    mb = (mask_base + cl) * P
                    nc.scalar.activation(
                        attnT[:, 0:P], sc_ps[:, 0:P],
                        mybir.ActivationFunctionType.Exp, scale=SCALE,
                    )
                    nc.vector.tensor_mul(attnT[:, 0:P], attnT[:, 0:P],
                                         maskblk[:, mb:mb + P])
                    if ncols > P:
                        nc.scalar.activation(
                            attnT[:, P:ncols], sc_ps[:, P:ncols],
                            mybir.ActivationFunctionType.Exp, scale=SCALE,
                        )
                else:
                    nc.scalar.activation(
                        attnT[:, :ncols], sc_ps[:, :ncols],
                        mybir.ActivationFunctionType.Exp, scale=SCALE,
                    )
                if pend_gemm is not None:
                    emit_gemm(pend_gemm)
                    pend_gemm = None
                if staged is not None:
                    pend_gemm = emit_back(staged)
                staged = (g, tbase, c, nchunks, j0, ncols, attnT, cd_ps)
            if pend_gemm is not None:
                emit_gemm(pend_gemm)
            pend_gemm = emit_back(staged)
            if pend_gemm is not None:
                emit_gemm(pend_gemm)

    nc.compile()
    _nc_cache = nc
    return nc


def tile_of_block(b, r):
    """Global q-tile held by output block b on role r."""
    return (8 + 2 * b + r) if b < 4 else (2 * (b - 4) + r)


def host_inputs(query, Wq, Wv, Wk):
    """Build per-core input maps. query [B,S,D] f32; W* [D,U] f32."""
    wqT16 = np.ascontiguousarray(Wq.T).astype(BF16)
    wvT16 = np.ascontiguousarray(Wv.T).astype(BF16)
    wk16 = Wk.astype(BF16)

    p = np.arange(P)[:, None]   # kk within chunk
    f = np.arange(P)[None, :]   # q within tile
    tri = (p < f).astype(np.float32)        # diag block: kk < q valid
    ones_b = np.ones((P, P), np.float32)
    zeros_b = np.zeros((P, P), np.float32)

    masks = {}
    for r in range(2):
        blocks = []
        for g, (tbase, nchunks) in enumerate(GROUPS):
            for cl in range(8):
                # chunk c = tbase + cl, affected slot j = cl//2,
                # role tile t = tbase + 2*(cl//2) + r
                c = tbase + cl
                t = tbase + 2 * (cl // 2) + r
                if c < t:
                    blocks.append(ones_b)
                elif c == t:
                    blocks.append(tri)
                else:
                    blocks.append(zeros_b)
        masks[r] = np.concatenate(blocks, axis=1).astype(BF16)

    in_maps = []
    for core in range(8):
        b_, r = core // 2, core % 2
        xTb = np.ascontiguousarray(query[b_].T).astype(BF16)      # [D, S]
        cols = np.concatenate(
            [np.arange(P * tile_of_block(b, r), P * tile_of_block(b, r) + P)
             for b in range(NSLOT)]
        )
        qx = np.ascontiguousarray(xTb[:, cols])                   # [D, 1024]
        misc = np.zeros((P, 16), np.float32)
        misc[0, 0] = 0.0 if r == 0 else 1.0      # rsel0
        misc[0, 1] = (1.0 / S) if r == 0 else 0.0  # rscale
        if r == 0:
            misc[0, 8 + SPECIAL] = 1.0           # den fix for global row 0
        in_maps.append({
            "xT": xTb, "qx": qx,
            "wqT": wqT16, "wvT": wvT16, "wk": wk16,
            "maskblk": masks[r], "misc": misc,
        })
    return in_maps


def assemble_output(results):
    """results: list of 8 dicts with 'out' [1024, 512] bf16."""
    out = np.zeros((B, S, U), np.float32)
    for core in range(8):
        b_, r = core // 2, core % 2
        o = np.asarray(results[core]["out"], dtype=np.float32)
        for b in range(NSLOT):
            t = tile_of_block(b, r)
            out[b_, P * t:P * (t + 1), :] = o[P * b:P * (b + 1), :]
    return out


def run(query, Wq, Wv, Wk, **kwargs):
    nc = build_nc()
    in_maps = host_inputs(
        np.asarray(query, np.float32), np.asarray(Wq, np.float32),
        np.asarray(Wv, np.float32), np.asarray(Wk, np.float32),
    )
    res = bass_utils.run_bass_kernel_spmd(nc, in_maps, list(range(8)), **kwargs)
    return assemble_output(res.results), res


def kernel(query, Wq, Wv, Wk):
    out, _ = run(query, Wq, Wv, Wk)
    return out


if __name__ == "__main__":
    rng = np.random.default_rng(0)
    q = rng.standard_normal((B, S, D), dtype=np.float32)
    scale = np.sqrt(2.0 / (D + U)).astype(np.float32)
    Wq = rng.standard_normal((D, U), dtype=np.float32) * scale
    Wv = rng.standard_normal((D, U), dtype=np.float32) * scale
    Wk = rng.standard_normal((D, U), dtype=np.float32) * scale
    out = kernel(q, Wq, Wv, Wk)
    print(out.shape, out.dtype, np.abs(out).mean())


# revision 24
# speedup vs baseline: 1.0520x; 1.0483x over previous
"""Trainium2 Bass kernel v3 for nn_Attention_14190571946482.

Causal self-attention (diagonal masked too), with both projection folds:
  B[d',d]   = sum_u Wv[d',u] Wq[d,u]          (device, 16 mm)
  u_cT[d,k] = sum_d' B[d',d] xT[d',k]         (replaces kT; scores become
  scoreT[k,q] = u_cT . xq / sqrt(D)            x A x^T - qT projection gone)
  ctxdT[d,q] = sum_c x_c[k,d]^T attnT_c[k,q]  (context in the d-basis -
  out[q,u]  = ctxdT^T @ Wk / den               v projection gone)

x_c chunk tiles ([k, d] layout) are PE-transposed from xT during phase 1
rather than DMA'd: the folds cut phase-1 PE work below the 2-queue DMA
feed rate, so input bytes are the binding resource (a late arrival also
risks the >~3us PE gap that resets the p-state, measured ~+18us).

Phase 2 is software-pipelined: scores/exp for chunk c+1 are emitted ahead
of den/ctxdT for chunk c (absorbs the exp->den latency), and each
finished slot's Wk-GEMM trails one further chunk so its cds copies
(vector) never stall the in-order PE queue.

Sharding: 8 cores = 4 batches x 2 roles; role r owns tiles {2j+r}.
Per core 2 groups of 4 slots: G2 = tiles {8..15} (chunks 0..15) first,
then G1 = tiles {0..7} (chunks 0..7). Role-dependent structure is input
data (qx gather, mask blocks, misc columns); the program is
SPMD-identical. Row 0 (fully masked) is blended to mean(v) on the final
psum before normalize.
"""

import sys

sys.path.insert(0, "/opt/trn_rl_repo")

import numpy as np
import ml_dtypes

import concourse.bass as bass
import concourse.bacc as bacc
import concourse.mybir as mybir
from concourse.tile import TileContext
from concourse.masks import make_identity
from concourse import bass_utils

BF16 = ml_dtypes.bfloat16

B, S, D, U = 4, 2048, 512, 512
P = 128
SCALE = 1.0 / float(np.sqrt(np.float32(D)))
GROUPS = [(8, 16), (0, 8)]  # (tbase, nchunks): G2 first, then G1
NSLOT = 8                   # output blocks: b=0..3 G2 slots, 4..7 G1
SPECIAL = 4                 # G1 slot 0 holds tiles (0,1): row-0 blend
NWARM = 10

_nc_cache = None


def build_nc():
    global _nc_cache
    if _nc_cache is not None:
        return _nc_cache

    f32 = mybir.dt.float32
    bf16 = mybir.dt.bfloat16

    nc = bacc.Bacc()
    xT_d = nc.declare_dram_parameter("xT", [D, S], bf16, isOutput=False)
    qx_d = nc.declare_dram_parameter("qx", [D, NSLOT * P], bf16, isOutput=False)
    wqT_d = nc.declare_dram_parameter("wqT", [U, D], bf16, isOutput=False)
    wvT_d = nc.declare_dram_parameter("wvT", [U, D], bf16, isOutput=False)
    wk_d = nc.declare_dram_parameter("wk", [D, U], bf16, isOutput=False)
    # 16 frontier mask blocks [128,128]: G2 chunks 8..15, then G1 0..7.
    mm_d = nc.declare_dram_parameter("maskblk", [P, 16 * P], bf16, isOutput=False)
    # misc f32: [0,0] rsel0 (row-0 ctx factor), [0,1] rscale (1/S or 0),
    # cols 8..15: per-output-block sume column.
    ms_d = nc.declare_dram_parameter("misc", [P, 16], f32, isOutput=False)
    out_d = nc.declare_dram_parameter("out", [NSLOT * P, U], bf16, isOutput=True)

    with TileContext(nc) as tc:
        with (
            tc.tile_pool(name="cst", bufs=1) as cst,
            tc.tile_pool(name="work", bufs=6) as work,
            tc.tile_pool(name="small", bufs=8) as small,
            tc.tile_pool(name="psA", bufs=2, space="PSUM") as psA,
            tc.tile_pool(name="psT", bufs=1, space="PSUM") as psT,
            tc.tile_pool(name="psC", bufs=4, space="PSUM") as psC,
            tc.tile_pool(name="psD", bufs=1, space="PSUM") as psD,
        ):
            # ---- on-chip constants ----
            # warm-up operand: iota, NOT zeros — the power governor keys
            # on multiplier bit-toggling, and zero x zero never trips the
            # clock boost (measured: HAM start 25us with zeros)
            wu = cst.tile([P, 512], bf16, tag="wu")
            nc.gpsimd.iota(wu, pattern=[[1, 512]], base=1,
                           channel_multiplier=7,
                           allow_small_or_imprecise_dtypes=True)
            ones_c = cst.tile([P, 1], bf16, tag="ones")
            nc.gpsimd.memset(ones_c, 1.0)
            ident = cst.tile([P, P], bf16, tag="ident")
            make_identity(nc, ident)

            # ---- input DMAs on the two HW-DGE issue queues (~150-180GB/s
            # each; gpsimd DMA is the slow SWDGE path - do not use).
            # qx[:512] covers all of G2; qx[512:] is G1-only (~40us in).
            # misc gates the first den add (~28us), maskblk the first
            # frontier chunk (~28us). ----
            wvT_t = cst.tile([P, 4, D], bf16, tag="wvT")
            wqT_t = cst.tile([P, 4, D], bf16, tag="wqT")
            xT_t = cst.tile([P, 4, S], bf16, tag="xT")
            xT_r = xT_d.rearrange("(d p) s -> p d s", p=P)
            wk_t = cst.tile([P, 4, U], bf16, tag="wk")
            qx_t = cst.tile([P, 4, NSLOT * P], bf16, tag="qx")
            qx_r = qx_d.rearrange("(d p) s -> p d s", p=P)
            maskblk = cst.tile([P, 16 * P], bf16, tag="maskblk")
            misc = cst.tile([P, 16], f32, tag="misc")
            nc.sync.dma_start(out=wvT_t, in_=wvT_d.rearrange("(k p) d -> p k d", p=P))
            nc.scalar.dma_start(out=wqT_t, in_=wqT_d.rearrange("(k p) d -> p k d", p=P))
            nc.sync.dma_start(out=xT_t[:, :, 0:512], in_=xT_r[:, :, 0:512])
            nc.scalar.dma_start(out=xT_t[:, :, 512:1024], in_=xT_r[:, :, 512:1024])
            nc.sync.dma_start(out=xT_t[:, :, 1024:1536], in_=xT_r[:, :, 1024:1536])
            nc.scalar.dma_start(out=xT_t[:, :, 1536:2048], in_=xT_r[:, :, 1536:2048])
            nc.sync.dma_start(out=qx_t[:, :, 0:512], in_=qx_r[:, :, 0:512])
            nc.scalar.dma_start(out=wk_t, in_=wk_d.rearrange("(d p) u -> p d u", p=P))
            nc.sync.dma_start(out=misc, in_=ms_d[:, :])
            nc.sync.dma_start(out=maskblk, in_=mm_d[:, :])
            nc.scalar.dma_start(out=qx_t[:, :, 512:1024], in_=qx_r[:, :, 512:1024])

            # ---- PE warm-up: ramp the HAM clock while DMAs land (also
            # bridges the PE to the wvT/wqT arrival ~13us) ----
            dume = small.tile([1, 1], bf16, tag="dume")
            for w in range(NWARM):
                wups = psA.tile([P, 512], f32, tag="blk")
                nc.tensor.matmul(wups, lhsT=wu[:, :P], rhs=wu,
                                 start=True, stop=True)
                if w == 0:
                    # preload the scalar-engine exp table off-critical-path
                    nc.scalar.activation(
                        dume, wups[0:1, 0:1],
                        mybir.ActivationFunctionType.Exp, scale=SCALE)

            uT = [cst.tile([P, S], bf16, tag=f"uT{m}", name=f"uT{m}")
                  for m in range(4)]
            xo_t = cst.tile([P, 16, D], bf16, tag="xo")

            def emit_quad(c):
                tq = psT.tile([P, 4, P], bf16, tag="tq")
                for mm in range(4):
                    nc.tensor.transpose(
                        tq[:, mm, :], xT_t[:, mm, c * P:(c + 1) * P], ident)
                    dst = xo_t[:, c, mm * P:(mm + 1) * P]
                    if (c + mm) % 2 == 0:
                        nc.scalar.copy(dst, tq[:, mm, :])
                    else:
                        nc.vector.tensor_copy(dst, tq[:, mm, :])

            # ---- transposes of x chunks 0..3 ([k, d] layout): need only
            # the first xT slice, so they keep the PE busy (and the clock
            # governor ramping) while wvT/wqT are still in flight ----
            for c in range(4):
                emit_quad(c)

            # ---- B = Wv @ Wq^T, tiles [d' part, d free] ----
            B_sb = cst.tile([P, 4, D], bf16, tag="Bsb")
            for t in range(4):
                ps = psA.tile([P, 512], f32, tag="blk")
                for ku in range(4):
                    nc.tensor.matmul(
                        ps,
                        lhsT=wvT_t[:, ku, t * P:(t + 1) * P],
                        rhs=wqT_t[:, ku, :],
                        start=(ku == 0), stop=(ku == 3),
                    )
                if t % 2 == 0:
                    nc.vector.tensor_copy(B_sb[:, t, :], ps)
                else:
                    nc.scalar.copy(B_sb[:, t, :], ps)

            # ---- u_cT [d, s] per g-slice (follows the xT DMA) ----
            ci = 0
            for g in range(4):
                if g in (1, 2):
                    # DMA-jitter insurance: independent filler matmuls so a
                    # late xT slice never leaves the PE (and the clock
                    # governor) idle
                    for _ in range(2):
                        wf = psA.tile([P, 512], f32, tag="blk")
                        nc.tensor.matmul(wf, lhsT=wu[:, :P], rhs=wu,
                                         start=True, stop=True)
                for m in range(4):
                    ps = psA.tile([P, 512], f32, tag="blk")
                    for t in range(4):
                        nc.tensor.matmul(
                            ps,
                            lhsT=B_sb[:, t, m * P:(m + 1) * P],
                            rhs=xT_t[:, t, g * 512:(g + 1) * 512],
                            start=(t == 0), stop=(t == 3),
                        )
                    dst = uT[m][:, g * 512:(g + 1) * 512]
                    if ci % 2 == 0:
                        nc.vector.tensor_copy(dst, ps)
                    else:
                        nc.scalar.copy(dst, ps)
                    ci += 1
                    # transpose quad for chunk c = 4g+m (g0's ran pre-B);
                    # its copies drain during the next m-iteration
                    if g > 0:
                        emit_quad(4 * g + m)

            # ---- phase 2: transposed-score attention, d-basis context ----
            vm_sb = cst.tile([1, 512], f32, tag="vm_sb")
            # cols 0..7: slot denominators; cols 8..11: per-d-tile column
            # sums of x (for mean-v), accumulated like den via ones-matmuls
            den_t = psD.tile([P, 12], f32, tag="dent", name="dent")
            nc.vector.memset(den_t, 0.0)

            def emit_back(st):
                """den + ctxdT (+slot-finish prep) for a staged chunk."""
                g, tbase, c, nchunks, j0, ncols, attnT, cd_ps = st
                cl = c - tbase
                fin = (cl % 2 == 1) and cl >= 0
                jf = cl // 2 if fin else -1
                js = list(range(j0, 4))
                if cl >= 0:
                    js = js[1:] + js[:1]  # masked slot j0 last
                for j in js:
                    b = 4 * g + j
                    blk = attnT[:, (j - j0) * P:(j - j0 + 1) * P]
                    nc.tensor.matmul(den_t[:, b:b + 1], lhsT=blk,
                                     rhs=ones_c, start=False,
                                     stop=(j == jf),
                                     skip_group_check=True)
                for m in range(4):
                    nc.tensor.matmul(
                        cd_ps[m][:, j0 * P:512],
                        lhsT=xo_t[:, c, m * P:(m + 1) * P],
                        rhs=attnT[:, :ncols],
                        start=(c == 0), stop=(c == nchunks - 1),
                        skip_group_check=True,
                    )
                if g == 0:
                    for m in range(4):
                        nc.tensor.matmul(den_t[:, 8 + m:9 + m],
                                         lhsT=xo_t[:, c, m * P:(m + 1) * P],
                                         rhs=ones_c, start=False,
                                         stop=(c == nchunks - 1),
                                         skip_group_check=True)
                if not fin:
                    return None
                j = jf
                b = 4 * g + j
                den = small.tile([P, 1], f32, tag="den")
                nc.vector.tensor_add(den, den_t[:, b:b + 1],
                                     misc[:, 8 + b:9 + b])
                rcp = small.tile([P, 1], f32, tag="rcp")
                nc.vector.reciprocal(rcp, den)
                # cds copies on vector only: the scalar queue must stay
                # clear for the next chunks' exps
                cds = work.tile([P, 4, P], bf16, tag="cds")
                for m in range(4):
                    nc.vector.tensor_copy(cds[:, m, :],
                                          cd_ps[m][:, j * P:(j + 1) * P])
                return (b, cds, rcp)

            def emit_gemm(st):
                """Wk-GEMM + normalize + out DMA for a finished slot."""
                b, cds, rcp = st
                out_ps = psA.tile([P, 512], f32, tag="blk")
                for m in range(4):
                    nc.tensor.matmul(out_ps, lhsT=cds[:, m, :],
                                     rhs=wk_t[:, m, :],
                                     start=(m == 0), stop=(m == 3))
                if b == SPECIAL:
                    # row 0 of role 0 = mean(v): on psum f32
                    nc.vector.tensor_scalar_mul(
                        out_ps[0:1, :], out_ps[0:1, :], misc[0:1, 0:1])
                    nc.vector.tensor_add(out_ps[0:1, :], out_ps[0:1, :],
                                         vm_sb)
                ctx_sb = work.tile([P, 512], bf16, tag="ctxs")
                for hh in range(2):
                    nc.scalar.activation(
                        ctx_sb[:, hh * 256:(hh + 1) * 256],
                        out_ps[:, hh * 256:(hh + 1) * 256],
                        mybir.ActivationFunctionType.Copy,
                        scale=rcp)
                    nc.sync.dma_start(
                        out=out_d[b * P:(b + 1) * P,
                                  hh * 256:(hh + 1) * 256],
                        in_=ctx_sb[:, hh * 256:(hh + 1) * 256])

            # flat chunk stream across both groups: the staged/pending
            # pipeline carries over the G2->G1 boundary so the PE never
            # drains at the group switch
            cd_pool = {}
            staged = None
            pend_gemm = None
            chunk_stream = [(g, tbase, nchunks, c)
                            for g, (tbase, nchunks) in enumerate(GROUPS)
                            for c in range(nchunks)]
            for g, tbase, nchunks, c in chunk_stream:
                if g == 1 and c == 0:
                    # ---- mean-of-v (for the fully-masked global row 0):
                    # needed first at the SPECIAL slot (G1 cl=1). The
                    # x column sums accumulated on PE during G2 (vector
                    # reduces get hoisted by the Tile scheduler into the
                    # phase-1 stream where they delay the uT copies) ----
                    xs4 = small.tile([P, 4], bf16, tag="xs4")
                    nc.vector.tensor_copy(xs4, den_t[:, 8:12])
                    vm_ps = psA.tile([1, 512], f32, tag="blk")
                    for d in range(4):
                        nc.tensor.matmul(vm_ps, lhsT=xs4[:, d:d + 1],
                                         rhs=wk_t[:, d, :],
                                         start=(d == 0), stop=(d == 3))
                    nc.vector.tensor_scalar_mul(vm_sb, vm_ps, misc[0:1, 1:2])
                if c == 0:
                    cd_pool[g] = [psC.tile([P, 512], f32, tag="ctx",
                                           name=f"cd{g}_{m}")
                                  for m in range(4)]
                cd_ps = cd_pool[g]
                mask_base = 0 if g == 0 else 8
                j0 = max(0, (c - tbase) // 2)
                ncols = (4 - j0) * P
                qoff = g * 512 + j0 * P
                sc_ps = psA.tile([P, 512], f32, tag="blk")
                for m in range(4):
                    nc.tensor.matmul(
                        sc_ps[:, :ncols],
                        lhsT=uT[m][:, c * P:(c + 1) * P],
                        rhs=qx_t[:, m, qoff:qoff + ncols],
                        start=(m == 0), stop=(m == 3),
                    )
                attnT = work.tile([P, 512], bf16, tag="attnT")
                cl = c - tbase
                if cl >= 0:
                    # frontier chunk: the masked block is always the
                    # first live block (j == j0). Exp it first so the
                    # vector mask-mul overlaps the exp of the rest.
                    mb = (mask_base + cl) * P
                    nc.scalar.activation(
                        attnT[:, 0:P], sc_ps[:, 0:P],
                        mybir.ActivationFunctionType.Exp, scale=SCALE,
                    )
                    nc.vector.tensor_mul(attnT[:, 0:P], attnT[:, 0:P],
                                         maskblk[:, mb:mb + P])
                    if ncols > P:
                        nc.scalar.activation(
                            attnT[:, P:ncols], sc_ps[:, P:ncols],
                            mybir.ActivationFunctionType.Exp, scale=SCALE,
                        )
                else:
                    nc.scalar.activation(
                        attnT[:, :ncols], sc_ps[:, :ncols],
                        mybir.ActivationFunctionType.Exp, scale=SCALE,
                    )
                if pend_gemm is not None:
                    emit_gemm(pend_gemm)
                    pend_gemm = None
                if staged is not None:
                    pend_gemm = emit_back(staged)
                staged = (g, tbase, c, nchunks, j0, ncols, attnT, cd_ps)
            if pend_gemm is not None:
                emit_gemm(pend_gemm)
            pend_gemm = emit_back(staged)
            if pend_gemm is not None:
                emit_gemm(pend_gemm)

    nc.compile()
    _nc_cache = nc
    return nc


def tile_of_block(b, r):
    """Global q-tile held by output block b on role r."""
    return (8 + 2 * b + r) if b < 4 else (2 * (b - 4) + r)


def host_inputs(query, Wq, Wv, Wk):
    """Build per-core input maps. query [B,S,D] f32; W* [D,U] f32."""
    wqT16 = np.ascontiguousarray(Wq.T).astype(BF16)
    wvT16 = np.ascontiguousarray(Wv.T).astype(BF16)
    wk16 = Wk.astype(BF16)

    p = np.arange(P)[:, None]   # kk within chunk
    f = np.arange(P)[None, :]   # q within tile
    tri = (p < f).astype(np.float32)        # diag block: kk < q valid
    ones_b = np.ones((P, P), np.float32)
    zeros_b = np.zeros((P, P), np.float32)

    masks = {}
    for r in range(2):
        blocks = []
        for g, (tbase, nchunks) in enumerate(GROUPS):
            for cl in range(8):
                # chunk c = tbase + cl, affected slot j = cl//2,
                # role tile t = tbase + 2*(cl//2) + r
                c = tbase + cl
                t = tbase + 2 * (cl // 2) + r
                if c < t:
                    blocks.append(ones_b)
                elif c == t:
                    blocks.append(tri)
                else:
                    blocks.append(zeros_b)
        masks[r] = np.concatenate(blocks, axis=1).astype(BF16)

    in_maps = []
    for core in range(8):
        b_, r = core // 2, core % 2
        xTb = np.ascontiguousarray(query[b_].T).astype(BF16)      # [D, S]
        cols = np.concatenate(
            [np.arange(P * tile_of_block(b, r), P * tile_of_block(b, r) + P)
             for b in range(NSLOT)]
        )
        qx = np.ascontiguousarray(xTb[:, cols])                   # [D, 1024]
        misc = np.zeros((P, 16), np.float32)
        misc[0, 0] = 0.0 if r == 0 else 1.0      # rsel0
        misc[0, 1] = (1.0 / S) if r == 0 else 0.0  # rscale
        if r == 0:
            misc[0, 8 + SPECIAL] = 1.0           # den fix for global row 0
        in_maps.append({
            "xT": xTb, "qx": qx,
            "wqT": wqT16, "wvT": wvT16, "wk": wk16,
            "maskblk": masks[r], "misc": misc,
        })
    return in_maps


def assemble_output(results):
    """results: list of 8 dicts with 'out' [1024, 512] bf16."""
    out = np.zeros((B, S, U), np.float32)
    for core in range(8):
        b_, r = core // 2, core % 2
        o = np.asarray(results[core]["out"], dtype=np.float32)
        for b in range(NSLOT):
            t = tile_of_block(b, r)
            out[b_, P * t:P * (t + 1), :] = o[P * b:P * (b + 1), :]
    return out


def run(query, Wq, Wv, Wk, **kwargs):
    nc = build_nc()
    in_maps = host_inputs(
        np.asarray(query, np.float32), np.asarray(Wq, np.float32),
        np.asarray(Wv, np.float32), np.asarray(Wk, np.float32),
    )
    res = bass_utils.run_bass_kernel_spmd(nc, in_maps, list(range(8)), **kwargs)
    return assemble_output(res.results), res


def kernel(query, Wq, Wv, Wk):
    out, _ = run(query, Wq, Wv, Wk)
    return out


if __name__ == "__main__":
    rng = np.random.default_rng(0)
    q = rng.standard_normal((B, S, D), dtype=np.float32)
    scale = np.sqrt(2.0 / (D + U)).astype(np.float32)
    Wq = rng.standard_normal((D, U), dtype=np.float32) * scale
    Wv = rng.standard_normal((D, U), dtype=np.float32) * scale
    Wk = rng.standard_normal((D, U), dtype=np.float32) * scale
    out = kernel(q, Wq, Wv, Wk)
    print(out.shape, out.dtype, np.abs(out).mean())


# revision 25
# speedup vs baseline: 1.0798x; 1.0264x over previous
"""Trainium2 Bass kernel v3 for nn_Attention_14190571946482.

Causal self-attention (diagonal masked too), with both projection folds:
  B[d',d]   = sum_u Wv[d',u] Wq[d,u]          (device, 16 mm)
  u_cT[d,k] = sum_d' B[d',d] xT[d',k]         (replaces kT; scores become
  scoreT[k,q] = u_cT . xq / sqrt(D)            x A x^T - qT projection gone)
  ctxdT[d,q] = sum_c x_c[k,d]^T attnT_c[k,q]  (context in the d-basis -
  out[q,u]  = ctxdT^T @ Wk / den               v projection gone)

x_c chunk tiles ([k, d] layout) are PE-transposed from xT during phase 1
rather than DMA'd: the folds cut phase-1 PE work below the 2-queue DMA
feed rate, so input bytes are the binding resource (a late arrival also
risks the >~3us PE gap that resets the p-state, measured ~+18us).

Phase 2 is software-pipelined: scores/exp for chunk c+1 are emitted ahead
of den/ctxdT for chunk c (absorbs the exp->den latency), and each
finished slot's Wk-GEMM trails one further chunk so its cds copies
(vector) never stall the in-order PE queue.

Sharding: 8 cores = 4 batches x 2 roles; role r owns tiles {2j+r}.
Per core 2 groups of 4 slots: G2 = tiles {8..15} (chunks 0..15) first,
then G1 = tiles {0..7} (chunks 0..7). Role-dependent structure is input
data (qx gather, mask blocks, misc columns); the program is
SPMD-identical. Row 0 (fully masked) is blended to mean(v) on the final
psum before normalize.
"""

import sys

sys.path.insert(0, "/opt/trn_rl_repo")

import numpy as np
import ml_dtypes

import concourse.bass as bass
import concourse.bacc as bacc
import concourse.mybir as mybir
from concourse.tile import TileContext
from concourse.masks import make_identity
from concourse import bass_utils

BF16 = ml_dtypes.bfloat16

B, S, D, U = 4, 2048, 512, 512
P = 128
SCALE = 1.0 / float(np.sqrt(np.float32(D)))
GROUPS = [(8, 16), (0, 8)]  # (tbase, nchunks): G2 first, then G1
NSLOT = 8                   # output blocks: b=0..3 G2 slots, 4..7 G1
SPECIAL = 4                 # G1 slot 0 holds tiles (0,1): row-0 blend
NWARM = 10

_nc_cache = None


def build_nc():
    global _nc_cache
    if _nc_cache is not None:
        return _nc_cache

    f32 = mybir.dt.float32
    bf16 = mybir.dt.bfloat16

    nc = bacc.Bacc()
    xT_d = nc.declare_dram_parameter("xT", [D, S], bf16, isOutput=False)
    qx_d = nc.declare_dram_parameter("qx", [D, NSLOT * P], bf16, isOutput=False)
    wqT_d = nc.declare_dram_parameter("wqT", [U, D], bf16, isOutput=False)
    wvT_d = nc.declare_dram_parameter("wvT", [U, D], bf16, isOutput=False)
    wk_d = nc.declare_dram_parameter("wk", [D, U], bf16, isOutput=False)
    # 16 frontier mask blocks [128,128]: G2 chunks 8..15, then G1 0..7.
    mm_d = nc.declare_dram_parameter("maskblk", [P, 16 * P], bf16, isOutput=False)
    # misc f32: [0,0] rsel0 (row-0 ctx factor), [0,1] rscale (1/S or 0),
    # cols 8..15: per-output-block sume column.
    ms_d = nc.declare_dram_parameter("misc", [P, 16], f32, isOutput=False)
    out_d = nc.declare_dram_parameter("out", [NSLOT * P, U], bf16, isOutput=True)

    with TileContext(nc) as tc:
        with (
            tc.tile_pool(name="cst", bufs=1) as cst,
            tc.tile_pool(name="work", bufs=6) as work,
            tc.tile_pool(name="small", bufs=8) as small,
            tc.tile_pool(name="psA", bufs=2, space="PSUM") as psA,
            tc.tile_pool(name="psT", bufs=1, space="PSUM") as psT,
            tc.tile_pool(name="psC", bufs=4, space="PSUM") as psC,
            tc.tile_pool(name="psD", bufs=1, space="PSUM") as psD,
        ):
            # ---- on-chip constants ----
            # warm-up operand: iota, NOT zeros — the power governor keys
            # on multiplier bit-toggling, and zero x zero never trips the
            # clock boost (measured: HAM start 25us with zeros)
            wu = cst.tile([P, 512], bf16, tag="wu")
            nc.gpsimd.iota(wu, pattern=[[1, 512]], base=1,
                           channel_multiplier=7,
                           allow_small_or_imprecise_dtypes=True)
            ones_c = cst.tile([P, 1], bf16, tag="ones")
            nc.gpsimd.memset(ones_c, 1.0)
            ident = cst.tile([P, P], bf16, tag="ident")
            make_identity(nc, ident)

            # ---- input DMAs on the two HW-DGE issue queues (~150-180GB/s
            # each; gpsimd DMA is the slow SWDGE path - do not use).
            # qx[:512] covers all of G2; qx[512:] is G1-only (~40us in).
            # misc gates the first den add (~28us), maskblk the first
            # frontier chunk (~28us). ----
            wvT_t = cst.tile([P, 4, D], bf16, tag="wvT")
            wqT_t = cst.tile([P, 4, D], bf16, tag="wqT")
            xT_t = cst.tile([P, 4, S], bf16, tag="xT")
            xT_r = xT_d.rearrange("(d p) s -> p d s", p=P)
            wk_t = cst.tile([P, 4, U], bf16, tag="wk")
            qx_t = cst.tile([P, 4, NSLOT * P], bf16, tag="qx")
            qx_r = qx_d.rearrange("(d p) s -> p d s", p=P)
            maskblk = cst.tile([P, 16 * P], bf16, tag="maskblk")
            misc = cst.tile([P, 16], f32, tag="misc")
            nc.sync.dma_start(out=wvT_t, in_=wvT_d.rearrange("(k p) d -> p k d", p=P))
            nc.scalar.dma_start(out=wqT_t, in_=wqT_d.rearrange("(k p) d -> p k d", p=P))
            nc.sync.dma_start(out=xT_t[:, :, 0:512], in_=xT_r[:, :, 0:512])
            nc.scalar.dma_start(out=xT_t[:, :, 512:1024], in_=xT_r[:, :, 512:1024])
            nc.sync.dma_start(out=xT_t[:, :, 1024:1536], in_=xT_r[:, :, 1024:1536])
            nc.scalar.dma_start(out=xT_t[:, :, 1536:2048], in_=xT_r[:, :, 1536:2048])
            nc.sync.dma_start(out=qx_t[:, :, 0:512], in_=qx_r[:, :, 0:512])
            nc.scalar.dma_start(out=wk_t, in_=wk_d.rearrange("(d p) u -> p d u", p=P))
            nc.sync.dma_start(out=misc, in_=ms_d[:, :])
            nc.sync.dma_start(out=maskblk, in_=mm_d[:, :])
            nc.scalar.dma_start(out=qx_t[:, :, 512:1024], in_=qx_r[:, :, 512:1024])

            # ---- PE warm-up: ramp the HAM clock while DMAs land (also
            # bridges the PE to the wvT/wqT arrival ~13us) ----
            dume = small.tile([1, 1], bf16, tag="dume")
            for w in range(NWARM):
                wups = psA.tile([P, 512], f32, tag="blk")
                nc.tensor.matmul(wups, lhsT=wu[:, :P], rhs=wu,
                                 start=True, stop=True)
                if w == 0:
                    # preload the scalar-engine exp table off-critical-path
                    nc.scalar.activation(
                        dume, wups[0:1, 0:1],
                        mybir.ActivationFunctionType.Exp, scale=SCALE)

            uT = [cst.tile([P, S], bf16, tag=f"uT{m}", name=f"uT{m}")
                  for m in range(4)]
            xo_t = cst.tile([P, 16, D], bf16, tag="xo")

            def emit_quad(c):
                tq = psT.tile([P, 4, P], bf16, tag="tq")
                for mm in range(4):
                    nc.tensor.transpose(
                        tq[:, mm, :], xT_t[:, mm, c * P:(c + 1) * P], ident)
                    dst = xo_t[:, c, mm * P:(mm + 1) * P]
                    if (c + mm) % 2 == 0:
                        nc.scalar.copy(dst, tq[:, mm, :])
                    else:
                        nc.vector.tensor_copy(dst, tq[:, mm, :])

            # ---- transposes of x chunks 0..3 ([k, d] layout): need only
            # the first xT slice, so they keep the PE busy (and the clock
            # governor ramping) while wvT/wqT are still in flight ----
            for c in range(4):
                emit_quad(c)

            # ---- B = Wv @ Wq^T, tiles [d' part, d free] ----
            B_sb = cst.tile([P, 4, D], bf16, tag="Bsb")
            for t in range(4):
                ps = psA.tile([P, 512], f32, tag="blk")
                for ku in range(4):
                    nc.tensor.matmul(
                        ps,
                        lhsT=wvT_t[:, ku, t * P:(t + 1) * P],
                        rhs=wqT_t[:, ku, :],
                        start=(ku == 0), stop=(ku == 3),
                    )
                if t % 2 == 0:
                    nc.vector.tensor_copy(B_sb[:, t, :], ps)
                else:
                    nc.scalar.copy(B_sb[:, t, :], ps)

            # ---- u_cT [d, s] per g-slice (follows the xT DMA) ----
            ci = 0
            for g in range(4):
                for m in range(4):
                    ps = psA.tile([P, 512], f32, tag="blk")
                    for t in range(4):
                        nc.tensor.matmul(
                            ps,
                            lhsT=B_sb[:, t, m * P:(m + 1) * P],
                            rhs=xT_t[:, t, g * 512:(g + 1) * 512],
                            start=(t == 0), stop=(t == 3),
                        )
                    dst = uT[m][:, g * 512:(g + 1) * 512]
                    if ci % 2 == 0:
                        nc.vector.tensor_copy(dst, ps)
                    else:
                        nc.scalar.copy(dst, ps)
                    ci += 1
                    # transpose quad for chunk c = 4g+m (g0's ran pre-B);
                    # its copies drain during the next m-iteration
                    if g > 0:
                        emit_quad(4 * g + m)

            # ---- phase 2: transposed-score attention, d-basis context ----
            vm_sb = cst.tile([1, 512], f32, tag="vm_sb")
            # cols 0..7: slot denominators; cols 8..11: per-d-tile column
            # sums of x (for mean-v), accumulated like den via ones-matmuls
            den_t = psD.tile([P, 12], f32, tag="dent", name="dent")
            nc.vector.memset(den_t, 0.0)

            def emit_back(st):
                """den + ctxdT (+slot-finish prep) for a staged chunk."""
                g, tbase, c, nchunks, j0, ncols, attnT, cd_ps = st
                cl = c - tbase
                fin = (cl % 2 == 1) and cl >= 0
                jf = cl // 2 if fin else -1
                js = list(range(j0, 4))
                if cl >= 0:
                    js = js[1:] + js[:1]  # masked slot j0 last
                for j in js:
                    b = 4 * g + j
                    blk = attnT[:, (j - j0) * P:(j - j0 + 1) * P]
                    nc.tensor.matmul(den_t[:, b:b + 1], lhsT=blk,
                                     rhs=ones_c, start=False,
                                     stop=(j == jf),
                                     skip_group_check=True)
                for m in range(4):
                    nc.tensor.matmul(
                        cd_ps[m][:, j0 * P:512],
                        lhsT=xo_t[:, c, m * P:(m + 1) * P],
                        rhs=attnT[:, :ncols],
                        start=(c == 0), stop=(c == nchunks - 1),
                        skip_group_check=True,
                    )
                if g == 0:
                    for m in range(4):
                        nc.tensor.matmul(den_t[:, 8 + m:9 + m],
                                         lhsT=xo_t[:, c, m * P:(m + 1) * P],
                                         rhs=ones_c, start=False,
                                         stop=(c == nchunks - 1),
                                         skip_group_check=True)
                if not fin:
                    return None
                j = jf
                b = 4 * g + j
                den = small.tile([P, 1], f32, tag="den")
                nc.vector.tensor_add(den, den_t[:, b:b + 1],
                                     misc[:, 8 + b:9 + b])
                rcp = small.tile([P, 1], f32, tag="rcp")
                nc.vector.reciprocal(rcp, den)
                # cds copies on vector only: the scalar queue must stay
                # clear for the next chunks' exps
                cds = work.tile([P, 4, P], bf16, tag="cds")
                for m in range(4):
                    nc.vector.tensor_copy(cds[:, m, :],
                                          cd_ps[m][:, j * P:(j + 1) * P])
                return (b, cds, rcp)

            def emit_gemm(st):
                """Wk-GEMM + normalize + out DMA for a finished slot."""
                b, cds, rcp = st
                out_ps = psA.tile([P, 512], f32, tag="blk")
                for m in range(4):
                    nc.tensor.matmul(out_ps, lhsT=cds[:, m, :],
                                     rhs=wk_t[:, m, :],
                                     start=(m == 0), stop=(m == 3))
                if b == SPECIAL:
                    # row 0 of role 0 = mean(v): on psum f32
                    nc.vector.tensor_scalar_mul(
                        out_ps[0:1, :], out_ps[0:1, :], misc[0:1, 0:1])
                    nc.vector.tensor_add(out_ps[0:1, :], out_ps[0:1, :],
                                         vm_sb)
                ctx_sb = work.tile([P, 512], bf16, tag="ctxs")
                for hh in range(2):
                    nc.scalar.activation(
                        ctx_sb[:, hh * 256:(hh + 1) * 256],
                        out_ps[:, hh * 256:(hh + 1) * 256],
                        mybir.ActivationFunctionType.Copy,
                        scale=rcp)
                    nc.sync.dma_start(
                        out=out_d[b * P:(b + 1) * P,
                                  hh * 256:(hh + 1) * 256],
                        in_=ctx_sb[:, hh * 256:(hh + 1) * 256])

            # flat chunk stream across both groups: the staged/pending
            # pipeline carries over the G2->G1 boundary so the PE never
            # drains at the group switch
            cd_pool = {}
            staged = None
            pend_gemm = None
            chunk_stream = [(g, tbase, nchunks, c)
                            for g, (tbase, nchunks) in enumerate(GROUPS)
                            for c in range(nchunks)]
            for g, tbase, nchunks, c in chunk_stream:
                if g == 1 and c == 0:
                    # ---- mean-of-v (for the fully-masked global row 0):
                    # needed first at the SPECIAL slot (G1 cl=1). The
                    # x column sums accumulated on PE during G2 (vector
                    # reduces get hoisted by the Tile scheduler into the
                    # phase-1 stream where they delay the uT copies) ----
                    xs4 = small.tile([P, 4], bf16, tag="xs4")
                    nc.vector.tensor_copy(xs4, den_t[:, 8:12])
                    vm_ps = psA.tile([1, 512], f32, tag="blk")
                    for d in range(4):
                        nc.tensor.matmul(vm_ps, lhsT=xs4[:, d:d + 1],
                                         rhs=wk_t[:, d, :],
                                         start=(d == 0), stop=(d == 3))
                    nc.vector.tensor_scalar_mul(vm_sb, vm_ps, misc[0:1, 1:2])
                if c == 0:
                    cd_pool[g] = [psC.tile([P, 512], f32, tag="ctx",
                                           name=f"cd{g}_{m}")
                                  for m in range(4)]
                cd_ps = cd_pool[g]
                mask_base = 0 if g == 0 else 8
                j0 = max(0, (c - tbase) // 2)
                ncols = (4 - j0) * P
                qoff = g * 512 + j0 * P
                sc_ps = psA.tile([P, 512], f32, tag="blk")
                for m in range(4):
                    nc.tensor.matmul(
                        sc_ps[:, :ncols],
                        lhsT=uT[m][:, c * P:(c + 1) * P],
                        rhs=qx_t[:, m, qoff:qoff + ncols],
                        start=(m == 0), stop=(m == 3),
                    )
                attnT = work.tile([P, 512], bf16, tag="attnT")
                cl = c - tbase
                if cl >= 0:
                    # frontier chunk: the masked block is always the
                    # first live block (j == j0). Exp it first so the
                    # vector mask-mul overlaps the exp of the rest.
                    mb = (mask_base + cl) * P
                    nc.scalar.activation(
                        attnT[:, 0:P], sc_ps[:, 0:P],
                        mybir.ActivationFunctionType.Exp, scale=SCALE,
                    )
                    nc.vector.tensor_mul(attnT[:, 0:P], attnT[:, 0:P],
                                         maskblk[:, mb:mb + P])
                    if ncols > P:
                        nc.scalar.activation(
                            attnT[:, P:ncols], sc_ps[:, P:ncols],
                            mybir.ActivationFunctionType.Exp, scale=SCALE,
                        )
                else:
                    nc.scalar.activation(
                        attnT[:, :ncols], sc_ps[:, :ncols],
                        mybir.ActivationFunctionType.Exp, scale=SCALE,
                    )
                if pend_gemm is not None:
                    emit_gemm(pend_gemm)
                    pend_gemm = None
                if staged is not None:
                    pend_gemm = emit_back(staged)
                staged = (g, tbase, c, nchunks, j0, ncols, attnT, cd_ps)
            if pend_gemm is not None:
                emit_gemm(pend_gemm)
            pend_gemm = emit_back(staged)
            if pend_gemm is not None:
                emit_gemm(pend_gemm)

    nc.compile()
    _nc_cache = nc
    return nc


def tile_of_block(b, r):
    """Global q-tile held by output block b on role r."""
    return (8 + 2 * b + r) if b < 4 else (2 * (b - 4) + r)


def host_inputs(query, Wq, Wv, Wk):
    """Build per-core input maps. query [B,S,D] f32; W* [D,U] f32."""
    wqT16 = np.ascontiguousarray(Wq.T).astype(BF16)
    wvT16 = np.ascontiguousarray(Wv.T).astype(BF16)
    wk16 = Wk.astype(BF16)

    p = np.arange(P)[:, None]   # kk within chunk
    f = np.arange(P)[None, :]   # q within tile
    tri = (p < f).astype(np.float32)        # diag block: kk < q valid
    ones_b = np.ones((P, P), np.float32)
    zeros_b = np.zeros((P, P), np.float32)

    masks = {}
    for r in range(2):
        blocks = []
        for g, (tbase, nchunks) in enumerate(GROUPS):
            for cl in range(8):
                # chunk c = tbase + cl, affected slot j = cl//2,
                # role tile t = tbase + 2*(cl//2) + r
                c = tbase + cl
                t = tbase + 2 * (cl // 2) + r
                if c < t:
                    blocks.append(ones_b)
                elif c == t:
                    blocks.append(tri)
                else:
                    blocks.append(zeros_b)
        masks[r] = np.concatenate(blocks, axis=1).astype(BF16)

    in_maps = []
    for core in range(8):
        b_, r = core // 2, core % 2
        xTb = np.ascontiguousarray(query[b_].T).astype(BF16)      # [D, S]
        cols = np.concatenate(
            [np.arange(P * tile_of_block(b, r), P * tile_of_block(b, r) + P)
             for b in range(NSLOT)]
        )
        qx = np.ascontiguousarray(xTb[:, cols])                   # [D, 1024]
        misc = np.zeros((P, 16), np.float32)
        misc[0, 0] = 0.0 if r == 0 else 1.0      # rsel0
        misc[0, 1] = (1.0 / S) if r == 0 else 0.0  # rscale
        if r == 0:
            misc[0, 8 + SPECIAL] = 1.0           # den fix for global row 0
        in_maps.append({
            "xT": xTb, "qx": qx,
            "wqT": wqT16, "wvT": wvT16, "wk": wk16,
            "maskblk": masks[r], "misc": misc,
        })
    return in_maps


def assemble_output(results):
    """results: list of 8 dicts with 'out' [1024, 512] bf16."""
    out = np.zeros((B, S, U), np.float32)
    for core in range(8):
        b_, r = core // 2, core % 2
        o = np.asarray(results[core]["out"], dtype=np.float32)
        for b in range(NSLOT):
            t = tile_of_block(b, r)
            out[b_, P * t:P * (t + 1), :] = o[P * b:P * (b + 1), :]
    return out


def run(query, Wq, Wv, Wk, **kwargs):
    nc = build_nc()
    in_maps = host_inputs(
        np.asarray(query, np.float32), np.asarray(Wq, np.float32),
        np.asarray(Wv, np.float32), np.asarray(Wk, np.float32),
    )
    res = bass_utils.run_bass_kernel_spmd(nc, in_maps, list(range(8)), **kwargs)
    return assemble_output(res.results), res


def kernel(query, Wq, Wv, Wk):
    out, _ = run(query, Wq, Wv, Wk)
    return out


if __name__ == "__main__":
    rng = np.random.default_rng(0)
    q = rng.standard_normal((B, S, D), dtype=np.float32)
    scale = np.sqrt(2.0 / (D + U)).astype(np.float32)
    Wq = rng.standard_normal((D, U), dtype=np.float32) * scale
    Wv = rng.standard_normal((D, U), dtype=np.float32) * scale
    Wk = rng.standard_normal((D, U), dtype=np.float32) * scale
    out = kernel(q, Wq, Wv, Wk)
    print(out.shape, out.dtype, np.abs(out).mean())


# revision 26
# speedup vs baseline: 1.0853x; 1.0052x over previous
"""Trainium2 Bass kernel v3 for nn_Attention_14190571946482.

Causal self-attention (diagonal masked too), with both projection folds:
  B[d',d]   = sum_u Wv[d',u] Wq[d,u]          (device, 16 mm)
  u_cT[d,k] = sum_d' B[d',d] xT[d',k]         (replaces kT; scores become
  scoreT[k,q] = u_cT . xq / sqrt(D)            x A x^T - qT projection gone)
  ctxdT[d,q] = sum_c x_c[k,d]^T attnT_c[k,q]  (context in the d-basis -
  out[q,u]  = ctxdT^T @ Wk / den               v projection gone)

x_c chunk tiles ([k, d] layout) are PE-transposed from xT during phase 1
rather than DMA'd: the folds cut phase-1 PE work below the 2-queue DMA
feed rate, so input bytes are the binding resource (a late arrival also
risks the >~3us PE gap that resets the p-state, measured ~+18us).

Phase 2 is software-pipelined: scores/exp for chunk c+1 are emitted ahead
of den/ctxdT for chunk c (absorbs the exp->den latency), and each
finished slot's Wk-GEMM trails one further chunk so its cds copies
(vector) never stall the in-order PE queue.

Sharding: 8 cores = 4 batches x 2 roles; role r owns tiles {2j+r}.
Per core 2 groups of 4 slots: G2 = tiles {8..15} (chunks 0..15) first,
then G1 = tiles {0..7} (chunks 0..7). Role-dependent structure is input
data (qx gather, mask blocks, misc columns); the program is
SPMD-identical. Row 0 (fully masked) is blended to mean(v) on the final
psum before normalize.
"""

import sys

sys.path.insert(0, "/opt/trn_rl_repo")

import numpy as np
import ml_dtypes

import concourse.bass as bass
import concourse.bacc as bacc
import concourse.mybir as mybir
from concourse.tile import TileContext
from concourse.masks import make_identity
from concourse import bass_utils

BF16 = ml_dtypes.bfloat16

B, S, D, U = 4, 2048, 512, 512
P = 128
SCALE = 1.0 / float(np.sqrt(np.float32(D)))
GROUPS = [(8, 16), (0, 8)]  # (tbase, nchunks): G2 first, then G1
NSLOT = 8                   # output blocks: b=0..3 G2 slots, 4..7 G1
SPECIAL = 4                 # G1 slot 0 holds tiles (0,1): row-0 blend
NWARM = 10

_nc_cache = None


def build_nc():
    global _nc_cache
    if _nc_cache is not None:
        return _nc_cache

    f32 = mybir.dt.float32
    bf16 = mybir.dt.bfloat16

    nc = bacc.Bacc()
    xT_d = nc.declare_dram_parameter("xT", [D, S], bf16, isOutput=False)
    qx_d = nc.declare_dram_parameter("qx", [D, NSLOT * P], bf16, isOutput=False)
    wqT_d = nc.declare_dram_parameter("wqT", [U, D], bf16, isOutput=False)
    wvT_d = nc.declare_dram_parameter("wvT", [U, D], bf16, isOutput=False)
    wk_d = nc.declare_dram_parameter("wk", [D, U], bf16, isOutput=False)
    # 16 frontier mask blocks [128,128]: G2 chunks 8..15, then G1 0..7.
    mm_d = nc.declare_dram_parameter("maskblk", [P, 16 * P], bf16, isOutput=False)
    # misc f32: [0,0] rsel0 (row-0 ctx factor), [0,1] rscale (1/S or 0),
    # cols 8..15: per-output-block sume column.
    ms_d = nc.declare_dram_parameter("misc", [P, 16], f32, isOutput=False)
    out_d = nc.declare_dram_parameter("out", [NSLOT * P, U], bf16, isOutput=True)

    with TileContext(nc) as tc:
        with (
            tc.tile_pool(name="cst", bufs=1) as cst,
            tc.tile_pool(name="work", bufs=6) as work,
            tc.tile_pool(name="small", bufs=8) as small,
            tc.tile_pool(name="psA", bufs=2, space="PSUM") as psA,
            tc.tile_pool(name="psT", bufs=1, space="PSUM") as psT,
            tc.tile_pool(name="psC", bufs=4, space="PSUM") as psC,
            tc.tile_pool(name="psD", bufs=1, space="PSUM") as psD,
        ):
            # ---- on-chip constants ----
            # warm-up operand: iota, NOT zeros — the power governor keys
            # on multiplier bit-toggling, and zero x zero never trips the
            # clock boost (measured: HAM start 25us with zeros)
            wu = cst.tile([P, 512], bf16, tag="wu")
            nc.gpsimd.iota(wu, pattern=[[1, 512]], base=1,
                           channel_multiplier=7,
                           allow_small_or_imprecise_dtypes=True)
            ones_c = cst.tile([P, 1], bf16, tag="ones")
            nc.gpsimd.memset(ones_c, 1.0)
            ident = cst.tile([P, P], bf16, tag="ident")
            make_identity(nc, ident)

            # ---- input DMAs on the two HW-DGE issue queues (~150-180GB/s
            # each; gpsimd DMA is the slow SWDGE path - do not use).
            # qx[:512] covers all of G2; qx[512:] is G1-only (~40us in).
            # misc gates the first den add (~28us), maskblk the first
            # frontier chunk (~28us). ----
            wvT_t = cst.tile([P, 4, D], bf16, tag="wvT")
            wqT_t = cst.tile([P, 4, D], bf16, tag="wqT")
            xT_t = cst.tile([P, 4, S], bf16, tag="xT")
            xT_r = xT_d.rearrange("(d p) s -> p d s", p=P)
            wk_t = cst.tile([P, 4, U], bf16, tag="wk")
            qx_t = cst.tile([P, 4, NSLOT * P], bf16, tag="qx")
            qx_r = qx_d.rearrange("(d p) s -> p d s", p=P)
            maskblk = cst.tile([P, 16 * P], bf16, tag="maskblk")
            misc = cst.tile([P, 16], f32, tag="misc")
            nc.sync.dma_start(out=wvT_t, in_=wvT_d.rearrange("(k p) d -> p k d", p=P))
            nc.scalar.dma_start(out=wqT_t, in_=wqT_d.rearrange("(k p) d -> p k d", p=P))
            nc.sync.dma_start(out=xT_t[:, :, 0:512], in_=xT_r[:, :, 0:512])
            nc.scalar.dma_start(out=xT_t[:, :, 512:1024], in_=xT_r[:, :, 512:1024])
            nc.sync.dma_start(out=xT_t[:, :, 1024:1536], in_=xT_r[:, :, 1024:1536])
            nc.scalar.dma_start(out=xT_t[:, :, 1536:2048], in_=xT_r[:, :, 1536:2048])
            nc.sync.dma_start(out=qx_t[:, :, 0:512], in_=qx_r[:, :, 0:512])
            nc.scalar.dma_start(out=wk_t, in_=wk_d.rearrange("(d p) u -> p d u", p=P))
            nc.sync.dma_start(out=misc, in_=ms_d[:, :])
            nc.sync.dma_start(out=maskblk, in_=mm_d[:, :])
            nc.scalar.dma_start(out=qx_t[:, :, 512:1024], in_=qx_r[:, :, 512:1024])

            # ---- PE warm-up: ramp the HAM clock while DMAs land (also
            # bridges the PE to the wvT/wqT arrival ~13us) ----
            dume = small.tile([1, 1], bf16, tag="dume")
            for w in range(NWARM):
                wups = psA.tile([P, 512], f32, tag="blk")
                nc.tensor.matmul(wups, lhsT=wu[:, :P], rhs=wu,
                                 start=True, stop=True)
                if w == 0:
                    # preload the scalar-engine exp table off-critical-path
                    nc.scalar.activation(
                        dume, wups[0:1, 0:1],
                        mybir.ActivationFunctionType.Exp, scale=SCALE)

            uT = [cst.tile([P, S], bf16, tag=f"uT{m}", name=f"uT{m}")
                  for m in range(4)]
            xo_t = cst.tile([P, 16, D], bf16, tag="xo")

            def emit_quad(c):
                tq = psT.tile([P, 4, P], bf16, tag="tq")
                for mm in range(4):
                    nc.tensor.transpose(
                        tq[:, mm, :], xT_t[:, mm, c * P:(c + 1) * P], ident)
                    dst = xo_t[:, c, mm * P:(mm + 1) * P]
                    if (c + mm) % 2 == 0:
                        nc.scalar.copy(dst, tq[:, mm, :])
                    else:
                        nc.vector.tensor_copy(dst, tq[:, mm, :])

            # ---- transposes of x chunks 0..3 ([k, d] layout): need only
            # the first xT slice, so they keep the PE busy (and the clock
            # governor ramping) while wvT/wqT are still in flight ----
            for c in range(4):
                emit_quad(c)

            # ---- B = Wv @ Wq^T, tiles [d' part, d free] ----
            B_sb = cst.tile([P, 4, D], bf16, tag="Bsb")
            for t in range(4):
                ps = psA.tile([P, 512], f32, tag="blk")
                for ku in range(4):
                    nc.tensor.matmul(
                        ps,
                        lhsT=wvT_t[:, ku, t * P:(t + 1) * P],
                        rhs=wqT_t[:, ku, :],
                        start=(ku == 0), stop=(ku == 3),
                    )
                if t % 2 == 0:
                    nc.vector.tensor_copy(B_sb[:, t, :], ps)
                else:
                    nc.scalar.copy(B_sb[:, t, :], ps)

            # ---- u_cT [d, s] per g-slice (follows the xT DMA) ----
            ci = 0
            for g in range(4):
                for m in range(4):
                    ps = psA.tile([P, 512], f32, tag="blk")
                    for t in range(4):
                        nc.tensor.matmul(
                            ps,
                            lhsT=B_sb[:, t, m * P:(m + 1) * P],
                            rhs=xT_t[:, t, g * 512:(g + 1) * 512],
                            start=(t == 0), stop=(t == 3),
                        )
                    dst = uT[m][:, g * 512:(g + 1) * 512]
                    if ci % 2 == 0:
                        nc.vector.tensor_copy(dst, ps)
                    else:
                        nc.scalar.copy(dst, ps)
                    ci += 1
                    # transpose quad for chunk c = 4g+m (g0's ran pre-B);
                    # its copies drain during the next m-iteration
                    if g > 0:
                        emit_quad(4 * g + m)

            # ---- phase 2: transposed-score attention, d-basis context ----
            vm_sb = cst.tile([1, 512], f32, tag="vm_sb")
            # cols 0..7: slot denominators; cols 8..11: per-d-tile column
            # sums of x (for mean-v), accumulated like den via ones-matmuls
            den_t = psD.tile([P, 12], f32, tag="dent", name="dent")
            nc.vector.memset(den_t, 0.0)

            def emit_back(st):
                """den + ctxdT (+slot-finish prep) for a staged chunk."""
                g, tbase, c, nchunks, j0, ncols, attnT, cd_ps = st
                cl = c - tbase
                fin = (cl % 2 == 1) and cl >= 0
                jf = cl // 2 if fin else -1
                js = list(range(j0, 4))
                if cl >= 0:
                    js = js[1:] + js[:1]  # masked slot j0 last
                for j in js:
                    b = 4 * g + j
                    blk = attnT[:, (j - j0) * P:(j - j0 + 1) * P]
                    nc.tensor.matmul(den_t[:, b:b + 1], lhsT=blk,
                                     rhs=ones_c, start=False,
                                     stop=(j == jf),
                                     skip_group_check=True)
                for m in range(4):
                    nc.tensor.matmul(
                        cd_ps[m][:, j0 * P:512],
                        lhsT=xo_t[:, c, m * P:(m + 1) * P],
                        rhs=attnT[:, :ncols],
                        start=(c == 0), stop=(c == nchunks - 1),
                        skip_group_check=True,
                    )
                if g == 0:
                    for m in range(4):
                        nc.tensor.matmul(den_t[:, 8 + m:9 + m],
                                         lhsT=xo_t[:, c, m * P:(m + 1) * P],
                                         rhs=ones_c, start=False,
                                         stop=(c == nchunks - 1),
                                         skip_group_check=True)
                if not fin:
                    return None
                j = jf
                b = 4 * g + j
                den = small.tile([P, 1], f32, tag="den")
                nc.vector.tensor_add(den, den_t[:, b:b + 1],
                                     misc[:, 8 + b:9 + b])
                rcp = small.tile([P, 1], f32, tag="rcp")
                nc.vector.reciprocal(rcp, den)
                # cds copies on vector only: the scalar queue must stay
                # clear for the next chunks' exps
                cds = work.tile([P, 4, P], bf16, tag="cds")
                for m in range(4):
                    nc.vector.tensor_copy(cds[:, m, :],
                                          cd_ps[m][:, j * P:(j + 1) * P])
                return (b, cds, rcp)

            def emit_gemm(st):
                """Wk-GEMM + normalize + out DMA for a finished slot."""
                b, cds, rcp = st
                out_ps = psA.tile([P, 512], f32, tag="blk")
                for m in range(4):
                    nc.tensor.matmul(out_ps, lhsT=cds[:, m, :],
                                     rhs=wk_t[:, m, :],
                                     start=(m == 0), stop=(m == 3))
                if b == SPECIAL:
                    # row 0 of role 0 = mean(v): on psum f32
                    nc.vector.tensor_scalar_mul(
                        out_ps[0:1, :], out_ps[0:1, :], misc[0:1, 0:1])
                    nc.vector.tensor_add(out_ps[0:1, :], out_ps[0:1, :],
                                         vm_sb)
                ctx_sb = work.tile([P, 512], bf16, tag="ctxs")
                nc.scalar.activation(
                    ctx_sb[:, 0:256], out_ps[:, 0:256],
                    mybir.ActivationFunctionType.Copy, scale=rcp)
                nc.sync.dma_start(out=out_d[b * P:(b + 1) * P, 0:256],
                                  in_=ctx_sb[:, 0:256])
                # second half normalized on vector (per-partition rcp):
                # halves the serial normalize chain and keeps the scalar
                # queue clear for the next exps
                nc.vector.tensor_scalar_mul(ctx_sb[:, 256:512],
                                            out_ps[:, 256:512], rcp)
                nc.sync.dma_start(out=out_d[b * P:(b + 1) * P, 256:512],
                                  in_=ctx_sb[:, 256:512])

            # flat chunk stream across both groups: the staged/pending
            # pipeline carries over the G2->G1 boundary so the PE never
            # drains at the group switch
            cd_pool = {}
            staged = None
            pend_gemm = None
            chunk_stream = [(g, tbase, nchunks, c)
                            for g, (tbase, nchunks) in enumerate(GROUPS)
                            for c in range(nchunks)]
            for g, tbase, nchunks, c in chunk_stream:
                if g == 1 and c == 0:
                    # ---- mean-of-v (for the fully-masked global row 0):
                    # needed first at the SPECIAL slot (G1 cl=1). The
                    # x column sums accumulated on PE during G2 (vector
                    # reduces get hoisted by the Tile scheduler into the
                    # phase-1 stream where they delay the uT copies) ----
                    xs4 = small.tile([P, 4], bf16, tag="xs4")
                    nc.vector.tensor_copy(xs4, den_t[:, 8:12])
                    vm_ps = psA.tile([1, 512], f32, tag="blk")
                    for d in range(4):
                        nc.tensor.matmul(vm_ps, lhsT=xs4[:, d:d + 1],
                                         rhs=wk_t[:, d, :],
                                         start=(d == 0), stop=(d == 3))
                    nc.vector.tensor_scalar_mul(vm_sb, vm_ps, misc[0:1, 1:2])
                if c == 0:
                    cd_pool[g] = [psC.tile([P, 512], f32, tag="ctx",
                                           name=f"cd{g}_{m}")
                                  for m in range(4)]
                cd_ps = cd_pool[g]
                mask_base = 0 if g == 0 else 8
                j0 = max(0, (c - tbase) // 2)
                ncols = (4 - j0) * P
                qoff = g * 512 + j0 * P
                sc_ps = psA.tile([P, 512], f32, tag="blk")
                for m in range(4):
                    nc.tensor.matmul(
                        sc_ps[:, :ncols],
                        lhsT=uT[m][:, c * P:(c + 1) * P],
                        rhs=qx_t[:, m, qoff:qoff + ncols],
                        start=(m == 0), stop=(m == 3),
                    )
                attnT = work.tile([P, 512], bf16, tag="attnT")
                cl = c - tbase
                if cl >= 0:
                    # frontier chunk: the masked block is always the
                    # first live block (j == j0). Exp it first so the
                    # vector mask-mul overlaps the exp of the rest.
                    mb = (mask_base + cl) * P
                    nc.scalar.activation(
                        attnT[:, 0:P], sc_ps[:, 0:P],
                        mybir.ActivationFunctionType.Exp, scale=SCALE,
                    )
                    nc.vector.tensor_mul(attnT[:, 0:P], attnT[:, 0:P],
                                         maskblk[:, mb:mb + P])
                    if ncols > P:
                        nc.scalar.activation(
                            attnT[:, P:ncols], sc_ps[:, P:ncols],
                            mybir.ActivationFunctionType.Exp, scale=SCALE,
                        )
                else:
                    nc.scalar.activation(
                        attnT[:, :ncols], sc_ps[:, :ncols],
                        mybir.ActivationFunctionType.Exp, scale=SCALE,
                    )
                if pend_gemm is not None:
                    emit_gemm(pend_gemm)
                    pend_gemm = None
                if staged is not None:
                    pend_gemm = emit_back(staged)
                staged = (g, tbase, c, nchunks, j0, ncols, attnT, cd_ps)
            if pend_gemm is not None:
                emit_gemm(pend_gemm)
            pend_gemm = emit_back(staged)
            if pend_gemm is not None:
                emit_gemm(pend_gemm)

    nc.compile()
    _nc_cache = nc
    return nc


def tile_of_block(b, r):
    """Global q-tile held by output block b on role r."""
    return (8 + 2 * b + r) if b < 4 else (2 * (b - 4) + r)


def host_inputs(query, Wq, Wv, Wk):
    """Build per-core input maps. query [B,S,D] f32; W* [D,U] f32."""
    wqT16 = np.ascontiguousarray(Wq.T).astype(BF16)
    wvT16 = np.ascontiguousarray(Wv.T).astype(BF16)
    wk16 = Wk.astype(BF16)

    p = np.arange(P)[:, None]   # kk within chunk
    f = np.arange(P)[None, :]   # q within tile
    tri = (p < f).astype(np.float32)        # diag block: kk < q valid
    ones_b = np.ones((P, P), np.float32)
    zeros_b = np.zeros((P, P), np.float32)

    masks = {}
    for r in range(2):
        blocks = []
        for g, (tbase, nchunks) in enumerate(GROUPS):
            for cl in range(8):
                # chunk c = tbase + cl, affected slot j = cl//2,
                # role tile t = tbase + 2*(cl//2) + r
                c = tbase + cl
                t = tbase + 2 * (cl // 2) + r
                if c < t:
                    blocks.append(ones_b)
                elif c == t:
                    blocks.append(tri)
                else:
                    blocks.append(zeros_b)
        masks[r] = np.concatenate(blocks, axis=1).astype(BF16)

    in_maps = []
    for core in range(8):
        b_, r = core // 2, core % 2
        xTb = np.ascontiguousarray(query[b_].T).astype(BF16)      # [D, S]
        cols = np.concatenate(
            [np.arange(P * tile_of_block(b, r), P * tile_of_block(b, r) + P)
             for b in range(NSLOT)]
        )
        qx = np.ascontiguousarray(xTb[:, cols])                   # [D, 1024]
        misc = np.zeros((P, 16), np.float32)
        misc[0, 0] = 0.0 if r == 0 else 1.0      # rsel0
        misc[0, 1] = (1.0 / S) if r == 0 else 0.0  # rscale
        if r == 0:
            misc[0, 8 + SPECIAL] = 1.0           # den fix for global row 0
        in_maps.append({
            "xT": xTb, "qx": qx,
            "wqT": wqT16, "wvT": wvT16, "wk": wk16,
            "maskblk": masks[r], "misc": misc,
        })
    return in_maps


def assemble_output(results):
    """results: list of 8 dicts with 'out' [1024, 512] bf16."""
    out = np.zeros((B, S, U), np.float32)
    for core in range(8):
        b_, r = core // 2, core % 2
        o = np.asarray(results[core]["out"], dtype=np.float32)
        for b in range(NSLOT):
            t = tile_of_block(b, r)
            out[b_, P * t:P * (t + 1), :] = o[P * b:P * (b + 1), :]
    return out


def run(query, Wq, Wv, Wk, **kwargs):
    nc = build_nc()
    in_maps = host_inputs(
        np.asarray(query, np.float32), np.asarray(Wq, np.float32),
        np.asarray(Wv, np.float32), np.asarray(Wk, np.float32),
    )
    res = bass_utils.run_bass_kernel_spmd(nc, in_maps, list(range(8)), **kwargs)
    return assemble_output(res.results), res


def kernel(query, Wq, Wv, Wk):
    out, _ = run(query, Wq, Wv, Wk)
    return out


if __name__ == "__main__":
    rng = np.random.default_rng(0)
    q = rng.standard_normal((B, S, D), dtype=np.float32)
    scale = np.sqrt(2.0 / (D + U)).astype(np.float32)
    Wq = rng.standard_normal((D, U), dtype=np.float32) * scale
    Wv = rng.standard_normal((D, U), dtype=np.float32) * scale
    Wk = rng.standard_normal((D, U), dtype=np.float32) * scale
    out = kernel(q, Wq, Wv, Wk)
    print(out.shape, out.dtype, np.abs(out).mean())
